# revision 13
# baseline (speedup 1.0000x reference)
"""Trainium2 Bass kernel for CustomizablePatchDominantGradientOrientation.

Pipeline per patch (32x32, fp32):
  sobel (replicate pad, [1,2,1]x[-1,0,1] separable; /8 dropped - the final
  angle is invariant to a global scale on (gx, gy, mag))
  mag = sqrt(gx^2+gy^2+eps'), theta = 2*atan(gy/(mag+gx))  (half-angle atan2)
  36-bin soft histogram via 18 dual-window custom-DVE passes: pass i
  (boundary b = (2i-18)*pi/36) emits in ONE instruction both
    W[2i]  = sum m*clamp(a-b, 0, d)    (in-pipe prefix-sum, streamed to a
                                        stride-0 out AP; final write = total)
    W2[2i] = sum m*clamp(a-b, 0, 2d)   (accumulator out_a path)
  with W[2i+1] = W2[2i] - W[2i]; hist[k] = W[k-1] - W[k] (bounded masks ->
  accumulation roundoff stays tiny).  Then circular [w0,w1,w2] smoothing,
  argmax, parabolic refinement -> angle.

Data parallel: B=32768 patches sharded over 8 NeuronCores (4096 each);
per core 32 tiles of [128 patches x 1024 pixels].  Layout is patch-major:
partitions = patches, free axis = pixels.
"""

import math

import numpy as np

NBINS = 36
PI = math.pi
PATCH = 32
HW = PATCH * PATCH
P = 128          # partitions (patches per tile)
N_CORES = 8
GROUP = 4        # tiles per ACT-table-set phase group
DELTA = PI / 36.0  # window width in atan units (theta = 2a, bin = 2pi/36)

_BUILD_CACHE = {}
_OPS_REGISTERED = {}


# --------------------------------------------------------------------------
# custom DVE ops
# --------------------------------------------------------------------------
def _register_custom_ops():
    """Register the fused ops at runtime (row assignment + sha pin, exactly
    what a source-level `OPS.append` would do).  DBLWIN_ANT's uop program is
    hand-assembled (dual outputs: in-pipe scan -> out port, accum -> out_a);
    its compiled DveOpSpec is pre-seeded into the compile cache."""
    if _OPS_REGISTERED:
        return _OPS_REGISTERED
    from operator import add as _op_add

    import concourse.dve_ops as dve_ops
    from concourse.dve_ops import DveOp, _COMPILE_CACHE
    from concourse.dve_spec import (
        Spec, Src0, Src1, C0, C1, C2, Zero, relu, minn, maxx, lower,
        _has_src1, Bin, _Placement, _State, _Stage, _assemble, COUNT_ONCE,
        PREV, sq as _sq,
    )
    from concourse.dve_uop import AluInp, AluOp, DveOpSpec, OutSel
    from concourse.dve_uop import DveOpSpec

    def _reg(name, spec):
        if name in dve_ops._SUB_OPCODE_FOR_NAME:
            for op in dve_ops.OPS:
                if op.name == name:
                    return op
        row = dve_ops._CUSTOM_DVE_ROW_BASE + len(dve_ops.OPS)
        assert row < 0x20, "custom-DVE row budget exhausted"
        dve_ops._SUB_OPCODE_FOR_NAME[name] = row
        shas = {}
        for ver in ("v3", "v4"):
            s = DveOpSpec(name=name, opcode=row, uops=lower(spec, ver=ver),
                          rd1_en=_has_src1(spec))
            shas[ver] = s.sha(ver)
        op = DveOp(name, spec, subdim=False, uops_sha=shas)
        dve_ops.OPS.append(op)
        dve_ops.CUSTOM_DVE_SPECS[name] = spec
        return op

    def _reg_hand(name, spec, uops):
        """Register with a hand-assembled v3 uop program (bypasses lower())."""
        if name in dve_ops._SUB_OPCODE_FOR_NAME:
            for op in dve_ops.OPS:
                if op.name == name:
                    return op
        row = dve_ops._CUSTOM_DVE_ROW_BASE + len(dve_ops.OPS)
        assert row < 0x20, "custom-DVE row budget exhausted"
        dve_ops._SUB_OPCODE_FOR_NAME[name] = row
        s3 = DveOpSpec(name=name, opcode=row, uops=uops, rd1_en=True)
        op = DveOp(name, spec, subdim=False, uops_sha={"v3": s3.sha("v3")})
        dve_ops.OPS.append(op)
        dve_ops.CUSTOM_DVE_SPECS[name] = spec
        _COMPILE_CACHE[(name, "v3")] = s3
        return op

    def _rsqrt_nr_ref(in0, in1, s0, s1, imm2):
        return ((s0 - in0 * in1 * in1 * s1) * in1).astype(np.float32)

    def _addmax_ref(in0, in1, s0, s1, imm2):
        return np.maximum(in0 + in1, s0).astype(np.float32)

    def _mul_sum_ref(in0, in1, s0, s1, imm2):
        o = (in0 * in1).astype(np.float32)
        return o, o.reshape(o.shape[0], -1).sum(axis=-1, keepdims=True)

    def _dblwin_ref(in0, in1, s0, s1, imm2):
        x = (in0 - np.float32(s0)).astype(np.float32)
        u = np.maximum(x, np.float32(0))
        pA = (np.minimum(u, np.float32(imm2)) * in1).astype(np.float32)
        pB = (np.minimum(u, np.float32(s1)) * in1).astype(np.float32)
        out = np.cumsum(pA.astype(np.float64), axis=-1).astype(np.float32)
        acc = (pB.reshape(pB.shape[0], -1).astype(np.float64)
               .sum(axis=-1, keepdims=True).astype(np.float32))
        return out, acc

    # z1 = (c0 - g2*z0^2*c1)*z0  (one Newton step toward 1/sqrt(g2))
    _OPS_REGISTERED["rsqrt_nr"] = _reg(
        "RSQRT_NR_ANT",
        Spec(body=(C0 - Src0 * _sq(Src1) * C1) * Src1,
             reference=_rsqrt_nr_ref))
    _OPS_REGISTERED["addmax"] = _reg(
        "ADD_MAX_ANT",
        Spec(body=maxx(Src0 + Src1, C0), reference=_addmax_ref))
    _OPS_REGISTERED["mulsum"] = _reg(
        "MUL_SUM_ANT",
        Spec(body=Src0 * Src1, accum=_op_add, reference=_mul_sum_ref))

    # -- DBLWIN_ANT: hand-assembled dual-window clamp-mask reduction --------
    # s0: x = a - swap0(b)          s4: scanA += pA        (lane3 -> out port)
    # s1: u = max(x, swap1(0))      s5: vB = min(lane2(u), swap5(2d))
    # s2: vA = min(u, swap2(d))     s6: pB = vB * m
    # s3: pA = vA * m               s7: accumB += pB       (out_a path)
    SWAP = AluInp.CURR_SWAP_OUT
    u_node = Bin(AluOp.MAX, Src0, Zero)       # capture-lane marker nodes
    scan_node = Bin(AluOp.ADD, Src0, Src1)
    pipeline = [
        _Stage(AluOp.SUBTRACT, Src0, SWAP),
        _Stage(AluOp.MAX, PREV, SWAP),
        _Stage(AluOp.MIN, PREV, SWAP),
        _Stage(AluOp.MULTIPLY, PREV, Src1),
        _Stage(AluOp.ADD, AluInp.CURR_ALU_OUT, PREV),
        _Stage(AluOp.MIN, AluInp.PREV_DELAY_2, SWAP),
        _Stage(AluOp.MULTIPLY, PREV, Src1),
        _Stage(AluOp.ADD, AluInp.CURR_ALU_OUT, PREV),
    ]
    p = _Placement(
        pipeline=pipeline, node_stage={},
        lane={Src0: 0, Src1: 1, u_node: 2, scan_node: 3},
        out_sel=OutSel.DELAY_3, accum_stage=7, captures=[(2, 2), (5, 3)],
    )
    latch_p = _Placement(
        pipeline=[_Stage(AluOp.BYPASS, PREV)] * 8, node_stage={},
        lane={C0: 0, Zero: 1, C2: 2, C1: 3},
        out_sel=OutSel.ALU_OUT, accum_stage=None, captures=[],
    )
    latch_ov = {
        0: _Stage(AluOp.BYPASS, C0, C0, swap=True),
        1: _Stage(AluOp.BYPASS, Zero, Zero, swap=True),
        2: _Stage(AluOp.BYPASS, C2, C2, swap=True),
        4: _Stage(AluOp.BYPASS, Zero, Zero, swap=True),
        5: _Stage(AluOp.BYPASS, C1, C1, swap=True),
        7: _Stage(AluOp.BYPASS, Zero, Zero, swap=True),
    }
    uops = [
        _assemble(_State(placement=latch_p, consume=(False, False),
                         overrides=latch_ov, trigger=COUNT_ONCE, repeat=1,
                         next=(1, 0, 0), write_out=False)),
        _assemble(_State(placement=p, consume=(False, False),
                         overrides={4: _Stage(AluOp.BYPASS, SWAP),
                                    7: _Stage(AluOp.BYPASS, SWAP)},
                         trigger=COUNT_ONCE, repeat=1, next=(2, 0, 0),
                         write_out=False)),
        _assemble(_State(placement=p, consume=(True, True))),
    ]
    _OPS_REGISTERED["dblwin"] = _reg_hand(
        "DBLWIN_ANT",
        Spec(body=minn(maxx(Src0 - C0, Zero), C2) * Src1, accum=_op_add,
             reference=_dblwin_ref),
        uops)
    return _OPS_REGISTERED


# --------------------------------------------------------------------------
# kernel build
# --------------------------------------------------------------------------
def _build(b_core, smooth_w, wk_is_ones):
    import concourse.bacc as bacc
    import concourse.mybir as mybir
    from concourse.tile import TileContext
    from concourse.bass import broadcast_tensor_aps

    ops = _register_custom_ops()
    RSQRT_NR, ADDMAX = ops["rsqrt_nr"], ops["addmax"]
    MULSUM, DBLWIN = ops["mulsum"], ops["dblwin"]

    f32 = mybir.dt.float32
    Alu = mybir.AluOpType
    Act = mybir.ActivationFunctionType

    n_tiles = b_core // P
    assert b_core % P == 0
    w0, w1, w2 = (float(x) for x in smooth_w)

    nc = bacc.Bacc(None, target_bir_lowering=False, debug=False)
    patch_in = nc.dram_tensor("patch", [b_core, HW], f32, kind="ExternalInput")
    # consts: iota36 repeated n_tiles times, then (iota36 - 64) repeated
    consts_in = nc.dram_tensor("consts", [P, 2 * n_tiles * NBINS], f32,
                               kind="ExternalInput")
    wk_in = None
    if not wk_is_ones:
        wk_in = nc.dram_tensor("wk", [P, HW], f32, kind="ExternalInput")
    out_t = nc.dram_tensor("angle", [b_core], f32, kind="ExternalOutput")

    with TileContext(nc) as tc:
        with tc.tile_pool(name="pool", bufs=2) as pool, \
             tc.tile_pool(name="persist", bufs=1) as pp:
            IOTA = pp.tile([P, n_tiles, NBINS], f32)
            IOTA64 = pp.tile([P, n_tiles, NBINS], f32)
            nc.sync.dma_start(IOTA[:], consts_in[:, 0:n_tiles * NBINS])
            nc.sync.dma_start(IOTA64[:], consts_in[:, n_tiles * NBINS:])
            WK = None
            if wk_in is not None:
                WK = pp.tile([P, HW], f32)
                nc.sync.dma_start(WK[:], wk_in[:])

            WS = pp.tile([P, n_tiles, 18], f32)   # W[2i]  (single window)
            WA = pp.tile([P, n_tiles, 18], f32)   # W2[2i] (double window)
            SMM = pp.tile([P, n_tiles, 1], f32)   # sum(m) per (patch, tile)
            EPS = pp.tile([P, 1], f32)            # sqrt bias (eps under root)
            nc.vector.memset(EPS[:], 6.4e-17)
            HEXT = pp.tile([P, n_tiles, NBINS + 2], f32)
            ANG = pp.tile([P, n_tiles], f32)

            n_groups = (n_tiles + GROUP - 1) // GROUP
            for g in range(n_groups):
                tiles = range(g * GROUP, min((g + 1) * GROUP, n_tiles))
                slot = {}
                # ---- phase A: sobel, magnitude (sqrt table set) ----
                for t in tiles:
                    s = t % GROUP
                    X = pool.tile([P, HW], f32, tag="x", bufs=3, name=f"x{t}")
                    nc.sync.dma_start(X[:], patch_in[t * P:(t + 1) * P, :])
                    X3 = X.rearrange("p (r c) -> p r c", c=PATCH)

                    SV = pool.tile([P, HW], f32, tag="sv", name=f"sv{t}")
                    # vertical [1,2,1] with replicate rows
                    nc.vector.scalar_tensor_tensor(
                        out=SV[:, 32:992], in0=X[:, 32:992], scalar=2.0,
                        in1=X[:, 0:960], op0=Alu.mult, op1=Alu.add)
                    nc.gpsimd.tensor_tensor(
                        SV[:, 32:992], SV[:, 32:992], X[:, 64:1024], Alu.add)
                    nc.vector.scalar_tensor_tensor(
                        out=SV[:, 0:32], in0=X[:, 0:32], scalar=3.0,
                        in1=X[:, 32:64], op0=Alu.mult, op1=Alu.add)
                    nc.vector.scalar_tensor_tensor(
                        out=SV[:, 992:1024], in0=X[:, 992:1024], scalar=3.0,
                        in1=X[:, 960:992], op0=Alu.mult, op1=Alu.add)
                    SV3 = SV.rearrange("p (r c) -> p r c", c=PATCH)

                    GX = pool.tile([P, HW], f32, tag=f"gx{s}", bufs=1,
                                   name=f"gx{t}")
                    GX3 = GX.rearrange("p (r c) -> p r c", c=PATCH)
                    # horizontal central difference with replicate cols
                    nc.vector.tensor_tensor(
                        GX3[:, :, 1:31], SV3[:, :, 2:32], SV3[:, :, 0:30],
                        Alu.subtract)
                    nc.vector.tensor_tensor(
                        GX3[:, :, 0:1], SV3[:, :, 1:2], SV3[:, :, 0:1],
                        Alu.subtract)
                    nc.vector.tensor_tensor(
                        GX3[:, :, 31:32], SV3[:, :, 31:32], SV3[:, :, 30:31],
                        Alu.subtract)

                    SH = pool.tile([P, HW], f32, tag="sh", name=f"sh{t}")
                    SH3 = SH.rearrange("p (r c) -> p r c", c=PATCH)
                    # horizontal [1,2,1] with replicate cols
                    nc.vector.scalar_tensor_tensor(
                        out=SH3[:, :, 1:31], in0=X3[:, :, 1:31], scalar=2.0,
                        in1=X3[:, :, 0:30], op0=Alu.mult, op1=Alu.add)
                    nc.vector.tensor_tensor(
                        SH3[:, :, 1:31], SH3[:, :, 1:31], X3[:, :, 2:32],
                        Alu.add)
                    nc.vector.scalar_tensor_tensor(
                        out=SH3[:, :, 0:1], in0=X3[:, :, 0:1], scalar=3.0,
                        in1=X3[:, :, 1:2], op0=Alu.mult, op1=Alu.add)
                    nc.vector.scalar_tensor_tensor(
                        out=SH3[:, :, 31:32], in0=X3[:, :, 31:32], scalar=3.0,
                        in1=X3[:, :, 30:31], op0=Alu.mult, op1=Alu.add)

                    GY = pool.tile([P, HW], f32, tag=f"gy{s}", bufs=1,
                                   name=f"gy{t}")
                    # vertical central difference with replicate rows
                    nc.gpsimd.tensor_tensor(
                        GY[:, 32:992], SH[:, 64:1024], SH[:, 0:960],
                        Alu.subtract)
                    nc.vector.tensor_tensor(
                        GY[:, 0:32], SH[:, 32:64], SH[:, 0:32], Alu.subtract)
                    nc.vector.tensor_tensor(
                        GY[:, 992:1024], SH[:, 992:1024], SH[:, 960:992],
                        Alu.subtract)

                    if WK is not None:
                        nc.vector.tensor_tensor(GX[:], GX[:], WK[:], Alu.mult)
                        nc.vector.tensor_tensor(GY[:], GY[:], WK[:], Alu.mult)

                    # g2 = gx^2 + gy^2 + eps  (eps scaled by 8^2 vs reference)
                    # sv/sh slots are dead here; reuse their tags for squares.
                    # Exact fp32 multiplies on GPSIMD (ACT Square is ~1e-5
                    # off, which poisons the magnitude beyond repair).
                    X2 = pool.tile([P, HW], f32, tag="sv", name=f"x2{t}")
                    Y2 = pool.tile([P, HW], f32, tag="sh", name=f"y2{t}")
                    nc.gpsimd.tensor_tensor(X2[:], GX[:], GX[:], Alu.mult)
                    nc.gpsimd.tensor_tensor(Y2[:], GY[:], GY[:], Alu.mult)
                    G2 = pool.tile([P, HW], f32, tag="g2", name=f"g2{t}")
                    nc.gpsimd.tensor_tensor(G2[:], X2[:], Y2[:], Alu.add)
                    M = pool.tile([P, HW], f32, tag=f"m{s}", bufs=1,
                                   name=f"m{t}")
                    # eps folded into the ACT free affine: sqrt(g2 + eps)
                    nc.scalar.activation(M[:], G2[:], Act.Sqrt, bias=EPS[:])
                    # one Newton step: m = g2 * nr(1/sqrt); sum(m) fused out
                    RC = pool.tile([P, HW], f32, tag="rc", name=f"rc{t}")
                    SC = pool.tile([P, HW], f32, tag="sc", name=f"sc{t}")
                    nc.vector.reciprocal_approx_fast(RC[:], M[:])
                    nc.vector._custom_dve(RSQRT_NR, out=SC[:], in0=G2[:],
                                          in1=RC[:], s0=1.5, s1=0.5)
                    nc.vector._custom_dve(MULSUM, out=M[:], in0=G2[:],
                                          in1=SC[:],
                                          accum_out=SMM[:, t, 0:1])
                    slot[t] = (GX, GY, M)

                # ---- phase B: orientation + histogram (sigmoid table set) --
                for t in tiles:
                    GX, GY, M = slot[t]
                    # d = max(m + gx, 1e-30): the clamp both avoids the
                    # recip(0)=NaN edge and pins rounding-negative d to the
                    # correct wrap side.
                    D = pool.tile([P, HW], f32, tag="g2", name=f"d{t}")
                    nc.vector._custom_dve(ADDMAX, out=D[:], in0=M[:],
                                          in1=GX[:], s0=1e-30)
                    RC = pool.tile([P, HW], f32, tag="rc", name=f"rcb{t}")
                    SC = pool.tile([P, HW], f32, tag="sc", name=f"scb{t}")
                    nc.vector.reciprocal_approx_fast(RC[:], D[:])
                    nc.gpsimd.tensor_tensor(SC[:], GY[:], RC[:], Alu.mult)
                    A = pool.tile([P, HW], f32, tag="a", name=f"a{t}")
                    nc.scalar.activation(A[:], SC[:], Act.Arctan)

                    # 18 dual-window passes: W[2i] via stride-0 scan out,
                    # W2[2i] via accumulator.
                    for i in range(18):
                        b = (2 * i - 18) * PI / 36.0
                        _, cell = broadcast_tensor_aps(A[:],
                                                       WS[:, t, i:i + 1])
                        nc.vector._custom_dve(
                            DBLWIN, out=cell, in0=A[:], in1=M[:],
                            s0=float(b), s1=float(2 * DELTA),
                            imm2=float(DELTA),
                            accum_out=WA[:, t, i:i + 1])

            # ---- tail: assemble hist, smoothing, argmax, refine (batched) --
            # odd bins 1,3..35 -> HEXT cols 2,4..36:  2*WS - WA
            nc.vector.scalar_tensor_tensor(
                out=HEXT[:, :, 2:38:2], in0=WS[:], scalar=2.0, in1=WA[:],
                op0=Alu.mult, op1=Alu.subtract)
            # WAS = WA - WS (kept in WA; WA dead after)
            nc.vector.tensor_tensor(WA[:], WA[:], WS[:], Alu.subtract)
            # even bins 2,4..34 -> HEXT cols 3,5..35: WAS[i] - WS[i+1], i=0..16
            nc.vector.tensor_tensor(
                HEXT[:, :, 3:37:2], WA[:, :, 0:17], WS[:, :, 1:18],
                Alu.subtract)
            # bin 0 -> col 1: delta*sum(m) - WS[0] + WAS[17]
            nc.vector.scalar_tensor_tensor(
                out=HEXT[:, :, 1:2], in0=SMM[:], scalar=float(DELTA),
                in1=WS[:, :, 0:1], op0=Alu.mult, op1=Alu.subtract)
            nc.vector.tensor_tensor(
                HEXT[:, :, 1:2], HEXT[:, :, 1:2], WA[:, :, 17:18], Alu.add)
            # wrap columns
            nc.vector.tensor_copy(HEXT[:, :, 0:1], HEXT[:, :, 36:37])
            nc.vector.tensor_copy(HEXT[:, :, 37:38], HEXT[:, :, 1:2])

            SM = pp.tile([P, n_tiles, NBINS], f32)
            nc.vector.tensor_scalar(SM[:], HEXT[:, :, 2:38], w2, None,
                                    Alu.mult)
            nc.vector.scalar_tensor_tensor(
                out=SM[:], in0=HEXT[:, :, 0:36], scalar=w0, in1=SM[:],
                op0=Alu.mult, op1=Alu.add)
            HS = pp.tile([P, n_tiles, NBINS], f32)
            nc.vector.scalar_tensor_tensor(
                out=HS[:], in0=HEXT[:, :, 1:37], scalar=w1, in1=SM[:],
                op0=Alu.mult, op1=Alu.add)

            VMAX = pp.tile([P, n_tiles, 1], f32)
            nc.vector.tensor_reduce(VMAX[:], HS[:], mybir.AxisListType.X,
                                    Alu.max)
            EQ = pp.tile([P, n_tiles, NBINS], f32)
            hs_b, vmax_b = broadcast_tensor_aps(HS[:], VMAX[:])
            nc.vector.tensor_tensor(EQ[:], hs_b, vmax_b, Alu.is_equal)
            nc.vector.tensor_tensor(EQ[:], EQ[:], IOTA64[:], Alu.mult)
            IDX = pp.tile([P, n_tiles, 1], f32)
            nc.vector.tensor_reduce(IDX[:], EQ[:], mybir.AxisListType.X,
                                    Alu.min)
            nc.vector.tensor_scalar(IDX[:], IDX[:], 64.0, None, Alu.add)

            def neighbor_value(shift, wrap_thr, wrap_add, nm):
                IDXN = pp.tile([P, n_tiles, 1], f32, name=f"idxn_{nm}")
                nc.vector.tensor_scalar(IDXN[:], IDX[:], float(shift), None,
                                        Alu.add)
                WADJ = pp.tile([P, n_tiles, 1], f32, name=f"wadj_{nm}")
                if wrap_add < 0:
                    nc.vector.tensor_scalar(WADJ[:], IDXN[:], wrap_thr,
                                            float(wrap_add), Alu.is_gt,
                                            Alu.mult)
                else:
                    nc.vector.tensor_scalar(WADJ[:], IDXN[:], wrap_thr,
                                            float(wrap_add), Alu.is_lt,
                                            Alu.mult)
                nc.vector.tensor_tensor(IDXN[:], IDXN[:], WADJ[:], Alu.add)
                DIF = pp.tile([P, n_tiles, NBINS], f32, name=f"dif_{nm}")
                iota_b, idxn_b = broadcast_tensor_aps(IOTA[:], IDXN[:])
                nc.vector.tensor_tensor(DIF[:], iota_b, idxn_b, Alu.subtract)
                nc.vector.tensor_scalar(DIF[:], DIF[:], 0.0, None,
                                        Alu.is_equal)
                nc.vector.tensor_tensor(DIF[:], DIF[:], HS[:], Alu.mult)
                V = pp.tile([P, n_tiles, 1], f32, name=f"v_{nm}")
                nc.vector.tensor_reduce(V[:], DIF[:], mybir.AxisListType.X,
                                        Alu.add)
                return V

            VP = neighbor_value(+1, 35.5, -36.0, "p")
            VM = neighbor_value(-1, -0.5, +36.0, "m")

            NUM = pp.tile([P, n_tiles, 1], f32)
            nc.vector.tensor_tensor(NUM[:], VP[:], VM[:], Alu.subtract)
            SUMN = pp.tile([P, n_tiles, 1], f32)
            nc.vector.tensor_tensor(SUMN[:], VP[:], VM[:], Alu.add)
            DEN = pp.tile([P, n_tiles, 1], f32)
            nc.vector.tensor_scalar(DEN[:], VMAX[:], 2.0, None, Alu.mult)
            nc.vector.tensor_tensor(DEN[:], DEN[:], SUMN[:], Alu.subtract)
            RECD = pp.tile([P, n_tiles, 1], f32)
            SCD = pp.tile([P, n_tiles, 1], f32)
            nc.vector.reciprocal_approx_accurate(RECD[:], DEN[:], SCD[:])
            REF = pp.tile([P, n_tiles, 1], f32)
            nc.vector.scalar_tensor_tensor(
                out=REF[:], in0=NUM[:], scalar=0.5, in1=RECD[:],
                op0=Alu.mult, op1=Alu.mult)
            nc.vector.tensor_tensor(REF[:], IDX[:], REF[:], Alu.add)
            nc.vector.tensor_scalar(ANG[:], REF[:, :, 0], -2.0 * PI / NBINS,
                                    PI, Alu.mult, Alu.add)

            out_view = out_t[:].rearrange("(t p) -> p t", p=P)
            nc.sync.dma_start(out_view, ANG[:])

    nc.compile()
    return nc


def _get_built(b_core, smooth_w, wk_is_ones):
    key = (b_core, tuple(float(x) for x in smooth_w), bool(wk_is_ones))
    if key not in _BUILD_CACHE:
        _BUILD_CACHE[key] = _build(b_core, smooth_w, wk_is_ones)
    return _BUILD_CACHE[key]


# --------------------------------------------------------------------------
# host entry point
# --------------------------------------------------------------------------
def kernel(patch, weight_kernel, smooth_w):
    from concourse import bass_utils

    patch = np.ascontiguousarray(np.asarray(patch, dtype=np.float32))
    weight_kernel = np.asarray(weight_kernel, dtype=np.float32)
    smooth_w = np.asarray(smooth_w, dtype=np.float32)

    B = patch.shape[0]
    assert B % (N_CORES * P) == 0, f"B={B} not divisible by {N_CORES * P}"
    b_core = B // N_CORES
    n_tiles = b_core // P

    wk_is_ones = bool(np.all(weight_kernel == 1.0))
    nc = _get_built(b_core, smooth_w, wk_is_ones)

    x = patch.reshape(N_CORES, b_core, HW)

    iota = np.tile(np.arange(NBINS, dtype=np.float32), n_tiles)
    consts_row = np.concatenate([iota, iota - 64.0]).astype(np.float32)
    consts = np.ascontiguousarray(
        np.broadcast_to(consts_row, (P, consts_row.size)))

    in_maps = []
    for i in range(N_CORES):
        m = {"patch": np.ascontiguousarray(x[i]), "consts": consts}
        if not wk_is_ones:
            m["wk"] = np.ascontiguousarray(
                np.broadcast_to(weight_kernel.reshape(-1), (P, HW)))
        in_maps.append(m)

    res = bass_utils.run_bass_kernel_spmd(nc, in_maps,
                                          core_ids=list(range(N_CORES)))
    out = np.concatenate([r["angle"] for r in res.results])
    return out.astype(np.float32)


# revision 15
# speedup vs baseline: 1.0164x; 1.0164x over previous
"""Trainium2 Bass kernel for CustomizablePatchDominantGradientOrientation.

Pipeline per patch (32x32, fp32):
  sobel (replicate pad, [1,2,1]x[-1,0,1] separable; /8 dropped - the final
  angle is invariant to a global scale on (gx, gy, mag))
  mag = sqrt(gx^2+gy^2+eps'), theta = 2*atan(gy/(mag+gx))  (half-angle atan2)
  36-bin soft histogram via 18 dual-window custom-DVE passes: pass i
  (boundary b = (2i-18)*pi/36) emits in ONE instruction both
    W[2i]  = sum m*clamp(a-b, 0, d)    (in-pipe prefix-sum, streamed to a
                                        stride-0 out AP; final write = total)
    W2[2i] = sum m*clamp(a-b, 0, 2d)   (accumulator out_a path)
  with W[2i+1] = W2[2i] - W[2i]; hist[k] = W[k-1] - W[k] (bounded masks ->
  accumulation roundoff stays tiny).  Then circular [w0,w1,w2] smoothing,
  argmax, parabolic refinement -> angle.

Data parallel: B=32768 patches sharded over 8 NeuronCores (4096 each);
per core 32 tiles of [128 patches x 1024 pixels].  Layout is patch-major:
partitions = patches, free axis = pixels.
"""

import math

import numpy as np

NBINS = 36
PI = math.pi
PATCH = 32
HW = PATCH * PATCH
P = 128          # partitions (patches per tile)
N_CORES = 8
GROUP = 4        # tiles per ACT-table-set phase group
DELTA = PI / 36.0  # window width in atan units (theta = 2a, bin = 2pi/36)

_BUILD_CACHE = {}
_OPS_REGISTERED = {}


# --------------------------------------------------------------------------
# custom DVE ops
# --------------------------------------------------------------------------
def _register_custom_ops():
    """Register the fused ops at runtime (row assignment + sha pin, exactly
    what a source-level `OPS.append` would do).  DBLWIN_ANT's uop program is
    hand-assembled (dual outputs: in-pipe scan -> out port, accum -> out_a);
    its compiled DveOpSpec is pre-seeded into the compile cache."""
    if _OPS_REGISTERED:
        return _OPS_REGISTERED
    from operator import add as _op_add

    import concourse.dve_ops as dve_ops
    from concourse.dve_ops import DveOp, _COMPILE_CACHE
    from concourse.dve_spec import (
        Spec, Src0, Src1, C0, C1, C2, Zero, relu, minn, maxx, lower,
        _has_src1, Bin, _Placement, _State, _Stage, _assemble, COUNT_ONCE,
        PREV, sq as _sq,
    )
    from concourse.dve_uop import AluInp, AluOp, DveOpSpec, OutSel
    from concourse.dve_uop import DveOpSpec

    def _reg(name, spec):
        if name in dve_ops._SUB_OPCODE_FOR_NAME:
            for op in dve_ops.OPS:
                if op.name == name:
                    return op
        row = dve_ops._CUSTOM_DVE_ROW_BASE + len(dve_ops.OPS)
        assert row < 0x20, "custom-DVE row budget exhausted"
        dve_ops._SUB_OPCODE_FOR_NAME[name] = row
        shas = {}
        for ver in ("v3", "v4"):
            s = DveOpSpec(name=name, opcode=row, uops=lower(spec, ver=ver),
                          rd1_en=_has_src1(spec))
            shas[ver] = s.sha(ver)
        op = DveOp(name, spec, subdim=False, uops_sha=shas)
        dve_ops.OPS.append(op)
        dve_ops.CUSTOM_DVE_SPECS[name] = spec
        return op

    def _reg_hand(name, spec, uops):
        """Register with a hand-assembled v3 uop program (bypasses lower())."""
        if name in dve_ops._SUB_OPCODE_FOR_NAME:
            for op in dve_ops.OPS:
                if op.name == name:
                    return op
        row = dve_ops._CUSTOM_DVE_ROW_BASE + len(dve_ops.OPS)
        assert row < 0x20, "custom-DVE row budget exhausted"
        dve_ops._SUB_OPCODE_FOR_NAME[name] = row
        s3 = DveOpSpec(name=name, opcode=row, uops=uops, rd1_en=True)
        op = DveOp(name, spec, subdim=False, uops_sha={"v3": s3.sha("v3")})
        dve_ops.OPS.append(op)
        dve_ops.CUSTOM_DVE_SPECS[name] = spec
        _COMPILE_CACHE[(name, "v3")] = s3
        return op

    def _rsqrt_nr_ref(in0, in1, s0, s1, imm2):
        return ((s0 - in0 * in1 * in1 * s1) * in1).astype(np.float32)

    def _addmax_ref(in0, in1, s0, s1, imm2):
        return np.maximum(in0 + in1, s0).astype(np.float32)

    def _mul_sum_ref(in0, in1, s0, s1, imm2):
        o = (in0 * in1).astype(np.float32)
        return o, o.reshape(o.shape[0], -1).sum(axis=-1, keepdims=True)

    def _dblwin_ref(in0, in1, s0, s1, imm2):
        x = (in0 - np.float32(s0)).astype(np.float32)
        u = np.maximum(x, np.float32(0))
        pA = (np.minimum(u, np.float32(imm2)) * in1).astype(np.float32)
        pB = (np.minimum(u, np.float32(s1)) * in1).astype(np.float32)
        out = np.cumsum(pA.astype(np.float64), axis=-1).astype(np.float32)
        acc = (pB.reshape(pB.shape[0], -1).astype(np.float64)
               .sum(axis=-1, keepdims=True).astype(np.float32))
        return out, acc

    # z1 = (c0 - g2*z0^2*c1)*z0  (one Newton step toward 1/sqrt(g2))
    _OPS_REGISTERED["rsqrt_nr"] = _reg(
        "RSQRT_NR_ANT",
        Spec(body=(C0 - Src0 * _sq(Src1) * C1) * Src1,
             reference=_rsqrt_nr_ref))
    _OPS_REGISTERED["addmax"] = _reg(
        "ADD_MAX_ANT",
        Spec(body=maxx(Src0 + Src1, C0), reference=_addmax_ref))
    _OPS_REGISTERED["mulsum"] = _reg(
        "MUL_SUM_ANT",
        Spec(body=Src0 * Src1, accum=_op_add, reference=_mul_sum_ref))

    # -- DBLWIN_ANT: hand-assembled dual-window clamp-mask reduction --------
    # s0: x = a - swap0(b)          s4: scanA += pA        (lane3 -> out port)
    # s1: u = max(x, swap1(0))      s5: vB = min(lane2(u), swap5(2d))
    # s2: vA = min(u, swap2(d))     s6: pB = vB * m
    # s3: pA = vA * m               s7: accumB += pB       (out_a path)
    SWAP = AluInp.CURR_SWAP_OUT
    u_node = Bin(AluOp.MAX, Src0, Zero)       # capture-lane marker nodes
    scan_node = Bin(AluOp.ADD, Src0, Src1)
    pipeline = [
        _Stage(AluOp.SUBTRACT, Src0, SWAP),
        _Stage(AluOp.MAX, PREV, SWAP),
        _Stage(AluOp.MIN, PREV, SWAP),
        _Stage(AluOp.MULTIPLY, PREV, Src1),
        _Stage(AluOp.ADD, AluInp.CURR_ALU_OUT, PREV),
        _Stage(AluOp.MIN, AluInp.PREV_DELAY_2, SWAP),
        _Stage(AluOp.MULTIPLY, PREV, Src1),
        _Stage(AluOp.ADD, AluInp.CURR_ALU_OUT, PREV),
    ]
    p = _Placement(
        pipeline=pipeline, node_stage={},
        lane={Src0: 0, Src1: 1, u_node: 2, scan_node: 3},
        out_sel=OutSel.DELAY_3, accum_stage=7, captures=[(2, 2), (5, 3)],
    )
    latch_p = _Placement(
        pipeline=[_Stage(AluOp.BYPASS, PREV)] * 8, node_stage={},
        lane={C0: 0, Zero: 1, C2: 2, C1: 3},
        out_sel=OutSel.ALU_OUT, accum_stage=None, captures=[],
    )
    latch_ov = {
        0: _Stage(AluOp.BYPASS, C0, C0, swap=True),
        1: _Stage(AluOp.BYPASS, Zero, Zero, swap=True),
        2: _Stage(AluOp.BYPASS, C2, C2, swap=True),
        4: _Stage(AluOp.BYPASS, Zero, Zero, swap=True),
        5: _Stage(AluOp.BYPASS, C1, C1, swap=True),
        7: _Stage(AluOp.BYPASS, Zero, Zero, swap=True),
    }
    uops = [
        _assemble(_State(placement=latch_p, consume=(False, False),
                         overrides=latch_ov, trigger=COUNT_ONCE, repeat=1,
                         next=(1, 0, 0), write_out=False)),
        _assemble(_State(placement=p, consume=(False, False),
                         overrides={4: _Stage(AluOp.BYPASS, SWAP),
                                    7: _Stage(AluOp.BYPASS, SWAP)},
                         trigger=COUNT_ONCE, repeat=1, next=(2, 0, 0),
                         write_out=False)),
        _assemble(_State(placement=p, consume=(True, True))),
    ]
    _OPS_REGISTERED["dblwin"] = _reg_hand(
        "DBLWIN_ANT",
        Spec(body=minn(maxx(Src0 - C0, Zero), C2) * Src1, accum=_op_add,
             reference=_dblwin_ref),
        uops)
    return _OPS_REGISTERED


# --------------------------------------------------------------------------
# kernel build
# --------------------------------------------------------------------------
def _build(b_core, smooth_w, wk_is_ones):
    import concourse.bacc as bacc
    import concourse.mybir as mybir
    from concourse.tile import TileContext
    from concourse.bass import broadcast_tensor_aps

    ops = _register_custom_ops()
    RSQRT_NR, ADDMAX = ops["rsqrt_nr"], ops["addmax"]
    MULSUM, DBLWIN = ops["mulsum"], ops["dblwin"]

    f32 = mybir.dt.float32
    Alu = mybir.AluOpType
    Act = mybir.ActivationFunctionType

    n_tiles = b_core // P
    assert b_core % P == 0
    w0, w1, w2 = (float(x) for x in smooth_w)

    nc = bacc.Bacc(None, target_bir_lowering=False, debug=False)
    patch_in = nc.dram_tensor("patch", [b_core, HW], f32, kind="ExternalInput")
    # consts: iota36 repeated n_tiles times, then (iota36 - 64) repeated
    consts_in = nc.dram_tensor("consts", [P, 2 * n_tiles * NBINS], f32,
                               kind="ExternalInput")
    wk_in = None
    if not wk_is_ones:
        wk_in = nc.dram_tensor("wk", [P, HW], f32, kind="ExternalInput")
    out_t = nc.dram_tensor("angle", [b_core], f32, kind="ExternalOutput")

    with TileContext(nc) as tc:
        with tc.tile_pool(name="pool", bufs=2) as pool, \
             tc.tile_pool(name="persist", bufs=1) as pp:
            IOTA = pp.tile([P, n_tiles, NBINS], f32)
            IOTA64 = pp.tile([P, n_tiles, NBINS], f32)
            nc.sync.dma_start(IOTA[:], consts_in[:, 0:n_tiles * NBINS])
            nc.sync.dma_start(IOTA64[:], consts_in[:, n_tiles * NBINS:])
            WK = None
            if wk_in is not None:
                WK = pp.tile([P, HW], f32)
                nc.sync.dma_start(WK[:], wk_in[:])

            WS = pp.tile([P, n_tiles, 18], f32)   # W[2i]  (single window)
            WA = pp.tile([P, n_tiles, 18], f32)   # W2[2i] (double window)
            SMM = pp.tile([P, n_tiles, 1], f32)   # sum(m) per (patch, tile)
            EPS = pp.tile([P, 1], f32)            # sqrt bias (eps under root)
            nc.vector.memset(EPS[:], 6.4e-17)
            HEXT = pp.tile([P, n_tiles, NBINS + 2], f32)
            ANG = pp.tile([P, n_tiles], f32)

            n_groups = (n_tiles + GROUP - 1) // GROUP
            for g in range(n_groups):
                tiles = range(g * GROUP, min((g + 1) * GROUP, n_tiles))
                slot = {}
                # ---- phase A: sobel, magnitude (sqrt table set) ----
                for t in tiles:
                    s = t % GROUP
                    X = pool.tile([P, HW], f32, tag="x", bufs=3, name=f"x{t}")
                    nc.sync.dma_start(X[:], patch_in[t * P:(t + 1) * P, :])
                    X3 = X.rearrange("p (r c) -> p r c", c=PATCH)

                    SV = pool.tile([P, HW], f32, tag="sv", name=f"sv{t}")
                    # vertical [1,2,1] with replicate rows
                    nc.vector.scalar_tensor_tensor(
                        out=SV[:, 32:992], in0=X[:, 32:992], scalar=2.0,
                        in1=X[:, 0:960], op0=Alu.mult, op1=Alu.add)
                    nc.vector.tensor_tensor(
                        SV[:, 32:992], SV[:, 32:992], X[:, 64:1024], Alu.add)
                    nc.vector.scalar_tensor_tensor(
                        out=SV[:, 0:32], in0=X[:, 0:32], scalar=3.0,
                        in1=X[:, 32:64], op0=Alu.mult, op1=Alu.add)
                    nc.vector.scalar_tensor_tensor(
                        out=SV[:, 992:1024], in0=X[:, 992:1024], scalar=3.0,
                        in1=X[:, 960:992], op0=Alu.mult, op1=Alu.add)
                    SV3 = SV.rearrange("p (r c) -> p r c", c=PATCH)

                    GX = pool.tile([P, HW], f32, tag=f"gx{s}", bufs=1,
                                   name=f"gx{t}")
                    GX3 = GX.rearrange("p (r c) -> p r c", c=PATCH)
                    # horizontal central difference with replicate cols
                    nc.vector.tensor_tensor(
                        GX3[:, :, 1:31], SV3[:, :, 2:32], SV3[:, :, 0:30],
                        Alu.subtract)
                    nc.vector.tensor_tensor(
                        GX3[:, :, 0:1], SV3[:, :, 1:2], SV3[:, :, 0:1],
                        Alu.subtract)
                    nc.vector.tensor_tensor(
                        GX3[:, :, 31:32], SV3[:, :, 31:32], SV3[:, :, 30:31],
                        Alu.subtract)

                    SH = pool.tile([P, HW], f32, tag="sh", name=f"sh{t}")
                    SH3 = SH.rearrange("p (r c) -> p r c", c=PATCH)
                    # horizontal [1,2,1] with replicate cols
                    nc.vector.scalar_tensor_tensor(
                        out=SH3[:, :, 1:31], in0=X3[:, :, 1:31], scalar=2.0,
                        in1=X3[:, :, 0:30], op0=Alu.mult, op1=Alu.add)
                    nc.vector.tensor_tensor(
                        SH3[:, :, 1:31], SH3[:, :, 1:31], X3[:, :, 2:32],
                        Alu.add)
                    nc.vector.scalar_tensor_tensor(
                        out=SH3[:, :, 0:1], in0=X3[:, :, 0:1], scalar=3.0,
                        in1=X3[:, :, 1:2], op0=Alu.mult, op1=Alu.add)
                    nc.vector.scalar_tensor_tensor(
                        out=SH3[:, :, 31:32], in0=X3[:, :, 31:32], scalar=3.0,
                        in1=X3[:, :, 30:31], op0=Alu.mult, op1=Alu.add)

                    GY = pool.tile([P, HW], f32, tag=f"gy{s}", bufs=1,
                                   name=f"gy{t}")
                    # vertical central difference with replicate rows
                    nc.vector.tensor_tensor(
                        GY[:, 32:992], SH[:, 64:1024], SH[:, 0:960],
                        Alu.subtract)
                    nc.vector.tensor_tensor(
                        GY[:, 0:32], SH[:, 32:64], SH[:, 0:32], Alu.subtract)
                    nc.vector.tensor_tensor(
                        GY[:, 992:1024], SH[:, 992:1024], SH[:, 960:992],
                        Alu.subtract)

                    if WK is not None:
                        nc.vector.tensor_tensor(GX[:], GX[:], WK[:], Alu.mult)
                        nc.vector.tensor_tensor(GY[:], GY[:], WK[:], Alu.mult)

                    # g2 = gx^2 + gy^2 + eps  (eps scaled by 8^2 vs reference)
                    # sv/sh slots are dead here; reuse their tags for squares.
                    # Exact fp32 multiplies on GPSIMD (ACT Square is ~1e-5
                    # off, which poisons the magnitude beyond repair).
                    X2 = pool.tile([P, HW], f32, tag="sv", name=f"x2{t}")
                    Y2 = pool.tile([P, HW], f32, tag="sh", name=f"y2{t}")
                    nc.gpsimd.tensor_tensor(X2[:], GX[:], GX[:], Alu.mult)
                    nc.gpsimd.tensor_tensor(Y2[:], GY[:], GY[:], Alu.mult)
                    G2 = pool.tile([P, HW], f32, tag="g2", name=f"g2{t}")
                    nc.gpsimd.tensor_tensor(G2[:], X2[:], Y2[:], Alu.add)
                    M = pool.tile([P, HW], f32, tag=f"m{s}", bufs=1,
                                   name=f"m{t}")
                    # eps folded into the ACT free affine: sqrt(g2 + eps)
                    nc.scalar.activation(M[:], G2[:], Act.Sqrt, bias=EPS[:])
                    # one Newton step: m = g2 * nr(1/sqrt); sum(m) fused out
                    RC = pool.tile([P, HW], f32, tag="rc", name=f"rc{t}")
                    SC = pool.tile([P, HW], f32, tag="sc", name=f"sc{t}")
                    nc.vector.reciprocal_approx_fast(RC[:], M[:])
                    nc.vector._custom_dve(RSQRT_NR, out=SC[:], in0=G2[:],
                                          in1=RC[:], s0=1.5, s1=0.5)
                    nc.vector._custom_dve(MULSUM, out=M[:], in0=G2[:],
                                          in1=SC[:],
                                          accum_out=SMM[:, t, 0:1])
                    slot[t] = (GX, GY, M)

                # ---- phase B: orientation + histogram (sigmoid table set) --
                for t in tiles:
                    GX, GY, M = slot[t]
                    # d = max(m + gx, 1e-30): the clamp both avoids the
                    # recip(0)=NaN edge and pins rounding-negative d to the
                    # correct wrap side.
                    D = pool.tile([P, HW], f32, tag="g2", name=f"d{t}")
                    nc.vector._custom_dve(ADDMAX, out=D[:], in0=M[:],
                                          in1=GX[:], s0=1e-30)
                    RC = pool.tile([P, HW], f32, tag="rc", name=f"rcb{t}")
                    SC = pool.tile([P, HW], f32, tag="sc", name=f"scb{t}")
                    nc.vector.reciprocal_approx_fast(RC[:], D[:])
                    nc.gpsimd.tensor_tensor(SC[:], GY[:], RC[:], Alu.mult)
                    A = pool.tile([P, HW], f32, tag="a", name=f"a{t}")
                    nc.scalar.activation(A[:], SC[:], Act.Arctan)

                    # 18 dual-window passes: W[2i] via stride-0 scan out,
                    # W2[2i] via accumulator.
                    for i in range(18):
                        b = (2 * i - 18) * PI / 36.0
                        _, cell = broadcast_tensor_aps(A[:],
                                                       WS[:, t, i:i + 1])
                        nc.vector._custom_dve(
                            DBLWIN, out=cell, in0=A[:], in1=M[:],
                            s0=float(b), s1=float(2 * DELTA),
                            imm2=float(DELTA),
                            accum_out=WA[:, t, i:i + 1])

            # ---- tail: assemble hist, smoothing, argmax, refine (batched) --
            # odd bins 1,3..35 -> HEXT cols 2,4..36:  2*WS - WA
            nc.vector.scalar_tensor_tensor(
                out=HEXT[:, :, 2:38:2], in0=WS[:], scalar=2.0, in1=WA[:],
                op0=Alu.mult, op1=Alu.subtract)
            # WAS = WA - WS (kept in WA; WA dead after)
            nc.vector.tensor_tensor(WA[:], WA[:], WS[:], Alu.subtract)
            # even bins 2,4..34 -> HEXT cols 3,5..35: WAS[i] - WS[i+1], i=0..16
            nc.vector.tensor_tensor(
                HEXT[:, :, 3:37:2], WA[:, :, 0:17], WS[:, :, 1:18],
                Alu.subtract)
            # bin 0 -> col 1: delta*sum(m) - WS[0] + WAS[17]
            nc.vector.scalar_tensor_tensor(
                out=HEXT[:, :, 1:2], in0=SMM[:], scalar=float(DELTA),
                in1=WS[:, :, 0:1], op0=Alu.mult, op1=Alu.subtract)
            nc.vector.tensor_tensor(
                HEXT[:, :, 1:2], HEXT[:, :, 1:2], WA[:, :, 17:18], Alu.add)
            # wrap columns
            nc.vector.tensor_copy(HEXT[:, :, 0:1], HEXT[:, :, 36:37])
            nc.vector.tensor_copy(HEXT[:, :, 37:38], HEXT[:, :, 1:2])

            SM = pp.tile([P, n_tiles, NBINS], f32)
            nc.vector.tensor_scalar(SM[:], HEXT[:, :, 2:38], w2, None,
                                    Alu.mult)
            nc.vector.scalar_tensor_tensor(
                out=SM[:], in0=HEXT[:, :, 0:36], scalar=w0, in1=SM[:],
                op0=Alu.mult, op1=Alu.add)
            HS = pp.tile([P, n_tiles, NBINS], f32)
            nc.vector.scalar_tensor_tensor(
                out=HS[:], in0=HEXT[:, :, 1:37], scalar=w1, in1=SM[:],
                op0=Alu.mult, op1=Alu.add)

            VMAX = pp.tile([P, n_tiles, 1], f32)
            nc.vector.tensor_reduce(VMAX[:], HS[:], mybir.AxisListType.X,
                                    Alu.max)
            EQ = pp.tile([P, n_tiles, NBINS], f32)
            hs_b, vmax_b = broadcast_tensor_aps(HS[:], VMAX[:])
            nc.vector.tensor_tensor(EQ[:], hs_b, vmax_b, Alu.is_equal)
            nc.vector.tensor_tensor(EQ[:], EQ[:], IOTA64[:], Alu.mult)
            IDX = pp.tile([P, n_tiles, 1], f32)
            nc.vector.tensor_reduce(IDX[:], EQ[:], mybir.AxisListType.X,
                                    Alu.min)
            nc.vector.tensor_scalar(IDX[:], IDX[:], 64.0, None, Alu.add)

            def neighbor_value(shift, wrap_thr, wrap_add, nm):
                IDXN = pp.tile([P, n_tiles, 1], f32, name=f"idxn_{nm}")
                nc.vector.tensor_scalar(IDXN[:], IDX[:], float(shift), None,
                                        Alu.add)
                WADJ = pp.tile([P, n_tiles, 1], f32, name=f"wadj_{nm}")
                if wrap_add < 0:
                    nc.vector.tensor_scalar(WADJ[:], IDXN[:], wrap_thr,
                                            float(wrap_add), Alu.is_gt,
                                            Alu.mult)
                else:
                    nc.vector.tensor_scalar(WADJ[:], IDXN[:], wrap_thr,
                                            float(wrap_add), Alu.is_lt,
                                            Alu.mult)
                nc.vector.tensor_tensor(IDXN[:], IDXN[:], WADJ[:], Alu.add)
                DIF = pp.tile([P, n_tiles, NBINS], f32, name=f"dif_{nm}")
                iota_b, idxn_b = broadcast_tensor_aps(IOTA[:], IDXN[:])
                nc.vector.tensor_tensor(DIF[:], iota_b, idxn_b, Alu.subtract)
                nc.vector.tensor_scalar(DIF[:], DIF[:], 0.0, None,
                                        Alu.is_equal)
                nc.vector.tensor_tensor(DIF[:], DIF[:], HS[:], Alu.mult)
                V = pp.tile([P, n_tiles, 1], f32, name=f"v_{nm}")
                nc.vector.tensor_reduce(V[:], DIF[:], mybir.AxisListType.X,
                                        Alu.add)
                return V

            VP = neighbor_value(+1, 35.5, -36.0, "p")
            VM = neighbor_value(-1, -0.5, +36.0, "m")

            NUM = pp.tile([P, n_tiles, 1], f32)
            nc.vector.tensor_tensor(NUM[:], VP[:], VM[:], Alu.subtract)
            SUMN = pp.tile([P, n_tiles, 1], f32)
            nc.vector.tensor_tensor(SUMN[:], VP[:], VM[:], Alu.add)
            DEN = pp.tile([P, n_tiles, 1], f32)
            nc.vector.tensor_scalar(DEN[:], VMAX[:], 2.0, None, Alu.mult)
            nc.vector.tensor_tensor(DEN[:], DEN[:], SUMN[:], Alu.subtract)
            RECD = pp.tile([P, n_tiles, 1], f32)
            SCD = pp.tile([P, n_tiles, 1], f32)
            nc.vector.reciprocal_approx_accurate(RECD[:], DEN[:], SCD[:])
            REF = pp.tile([P, n_tiles, 1], f32)
            nc.vector.scalar_tensor_tensor(
                out=REF[:], in0=NUM[:], scalar=0.5, in1=RECD[:],
                op0=Alu.mult, op1=Alu.mult)
            nc.vector.tensor_tensor(REF[:], IDX[:], REF[:], Alu.add)
            nc.vector.tensor_scalar(ANG[:], REF[:, :, 0], -2.0 * PI / NBINS,
                                    PI, Alu.mult, Alu.add)

            out_view = out_t[:].rearrange("(t p) -> p t", p=P)
            nc.sync.dma_start(out_view, ANG[:])

    nc.compile()
    return nc


def _get_built(b_core, smooth_w, wk_is_ones):
    key = (b_core, tuple(float(x) for x in smooth_w), bool(wk_is_ones))
    if key not in _BUILD_CACHE:
        _BUILD_CACHE[key] = _build(b_core, smooth_w, wk_is_ones)
    return _BUILD_CACHE[key]


# --------------------------------------------------------------------------
# host entry point
# --------------------------------------------------------------------------
def kernel(patch, weight_kernel, smooth_w):
    from concourse import bass_utils

    patch = np.ascontiguousarray(np.asarray(patch, dtype=np.float32))
    weight_kernel = np.asarray(weight_kernel, dtype=np.float32)
    smooth_w = np.asarray(smooth_w, dtype=np.float32)

    B = patch.shape[0]
    assert B % (N_CORES * P) == 0, f"B={B} not divisible by {N_CORES * P}"
    b_core = B // N_CORES
    n_tiles = b_core // P

    wk_is_ones = bool(np.all(weight_kernel == 1.0))
    nc = _get_built(b_core, smooth_w, wk_is_ones)

    x = patch.reshape(N_CORES, b_core, HW)

    iota = np.tile(np.arange(NBINS, dtype=np.float32), n_tiles)
    consts_row = np.concatenate([iota, iota - 64.0]).astype(np.float32)
    consts = np.ascontiguousarray(
        np.broadcast_to(consts_row, (P, consts_row.size)))

    in_maps = []
    for i in range(N_CORES):
        m = {"patch": np.ascontiguousarray(x[i]), "consts": consts}
        if not wk_is_ones:
            m["wk"] = np.ascontiguousarray(
                np.broadcast_to(weight_kernel.reshape(-1), (P, HW)))
        in_maps.append(m)

    res = bass_utils.run_bass_kernel_spmd(nc, in_maps,
                                          core_ids=list(range(N_CORES)))
    out = np.concatenate([r["angle"] for r in res.results])
    return out.astype(np.float32)


# revision 18
# speedup vs baseline: 1.0718x; 1.0545x over previous
"""Trainium2 Bass kernel for CustomizablePatchDominantGradientOrientation.

Pipeline per patch (32x32, fp32):
  sobel (replicate pad, [1,2,1]x[-1,0,1] separable; /8 dropped - the final
  angle is invariant to a global scale on (gx, gy, mag))
  mag = sqrt(gx^2+gy^2+eps'), theta = 2*atan(gy/(mag+gx))  (half-angle atan2)
  36-bin soft histogram via 18 dual-window custom-DVE passes: pass i
  (boundary b = (2i-18)*pi/36) emits in ONE instruction both
    W[2i]  = sum m*clamp(a-b, 0, d)    (in-pipe prefix-sum, streamed to a
                                        stride-0 out AP; final write = total)
    W2[2i] = sum m*clamp(a-b, 0, 2d)   (accumulator out_a path)
  with W[2i+1] = W2[2i] - W[2i]; hist[k] = W[k-1] - W[k] (bounded masks ->
  accumulation roundoff stays tiny).  Then circular [w0,w1,w2] smoothing,
  argmax, parabolic refinement -> angle.

Data parallel: B=32768 patches sharded over 8 NeuronCores (4096 each);
per core 32 tiles of [128 patches x 1024 pixels].  Layout is patch-major:
partitions = patches, free axis = pixels.
"""

import math

import numpy as np

NBINS = 36
PI = math.pi
PATCH = 32
HW = PATCH * PATCH
P = 128          # partitions (patches per tile)
N_CORES = 8
GROUP = 4        # tiles per ACT-table-set phase group
DELTA = PI / 36.0  # window width in atan units (theta = 2a, bin = 2pi/36)

_BUILD_CACHE = {}
_OPS_REGISTERED = {}


# --------------------------------------------------------------------------
# custom DVE ops
# --------------------------------------------------------------------------
def _register_custom_ops():
    """Register the fused ops at runtime (row assignment + sha pin, exactly
    what a source-level `OPS.append` would do).  DBLWIN_ANT's uop program is
    hand-assembled (dual outputs: in-pipe scan -> out port, accum -> out_a);
    its compiled DveOpSpec is pre-seeded into the compile cache."""
    if _OPS_REGISTERED:
        return _OPS_REGISTERED
    from operator import add as _op_add

    import concourse.dve_ops as dve_ops
    from concourse.dve_ops import DveOp, _COMPILE_CACHE
    from concourse.dve_spec import (
        Spec, Src0, Src1, C0, C1, C2, Zero, relu, minn, maxx, lower,
        _has_src1, Bin, _Placement, _State, _Stage, _assemble, COUNT_ONCE,
        PREV, sq as _sq,
    )
    from concourse.dve_uop import AluInp, AluOp, DveOpSpec, OutSel
    from concourse.dve_uop import DveOpSpec

    def _reg(name, spec):
        if name in dve_ops._SUB_OPCODE_FOR_NAME:
            for op in dve_ops.OPS:
                if op.name == name:
                    return op
        row = dve_ops._CUSTOM_DVE_ROW_BASE + len(dve_ops.OPS)
        assert row < 0x20, "custom-DVE row budget exhausted"
        dve_ops._SUB_OPCODE_FOR_NAME[name] = row
        shas = {}
        for ver in ("v3", "v4"):
            s = DveOpSpec(name=name, opcode=row, uops=lower(spec, ver=ver),
                          rd1_en=_has_src1(spec))
            shas[ver] = s.sha(ver)
        op = DveOp(name, spec, subdim=False, uops_sha=shas)
        dve_ops.OPS.append(op)
        dve_ops.CUSTOM_DVE_SPECS[name] = spec
        return op

    def _reg_hand(name, spec, uops):
        """Register with a hand-assembled v3 uop program (bypasses lower())."""
        if name in dve_ops._SUB_OPCODE_FOR_NAME:
            for op in dve_ops.OPS:
                if op.name == name:
                    return op
        row = dve_ops._CUSTOM_DVE_ROW_BASE + len(dve_ops.OPS)
        assert row < 0x20, "custom-DVE row budget exhausted"
        dve_ops._SUB_OPCODE_FOR_NAME[name] = row
        s3 = DveOpSpec(name=name, opcode=row, uops=uops, rd1_en=True)
        op = DveOp(name, spec, subdim=False, uops_sha={"v3": s3.sha("v3")})
        dve_ops.OPS.append(op)
        dve_ops.CUSTOM_DVE_SPECS[name] = spec
        _COMPILE_CACHE[(name, "v3")] = s3
        return op

    def _rsqrt_nr_ref(in0, in1, s0, s1, imm2):
        return ((s0 - in0 * in1 * in1 * s1) * in1).astype(np.float32)

    def _addmax_ref(in0, in1, s0, s1, imm2):
        return np.maximum(in0 + in1, s0).astype(np.float32)

    def _mul_sum_ref(in0, in1, s0, s1, imm2):
        o = (in0 * in1).astype(np.float32)
        return o, o.reshape(o.shape[0], -1).sum(axis=-1, keepdims=True)

    def _dblwin_ref(in0, in1, s0, s1, imm2):
        x = (in0 - np.float32(s0)).astype(np.float32)
        u = np.maximum(x, np.float32(0))
        pA = (np.minimum(u, np.float32(imm2)) * in1).astype(np.float32)
        pB = (np.minimum(u, np.float32(s1)) * in1).astype(np.float32)
        out = np.cumsum(pA.astype(np.float64), axis=-1).astype(np.float32)
        acc = (pB.reshape(pB.shape[0], -1).astype(np.float64)
               .sum(axis=-1, keepdims=True).astype(np.float32))
        return out, acc

    # z1 = (c0 - g2*z0^2*c1)*z0  (one Newton step toward 1/sqrt(g2))
    _OPS_REGISTERED["rsqrt_nr"] = _reg(
        "RSQRT_NR_ANT",
        Spec(body=(C0 - Src0 * _sq(Src1) * C1) * Src1,
             reference=_rsqrt_nr_ref))
    _OPS_REGISTERED["addmax"] = _reg(
        "ADD_MAX_ANT",
        Spec(body=maxx(Src0 + Src1, C0), reference=_addmax_ref))
    _OPS_REGISTERED["mulsum"] = _reg(
        "MUL_SUM_ANT",
        Spec(body=Src0 * Src1, accum=_op_add, reference=_mul_sum_ref))

    # -- DBLWIN_ANT: hand-assembled dual-window clamp-mask reduction --------
    # s0: x = a - swap0(b)          s4: scanA += pA        (lane3 -> out port)
    # s1: u = max(x, swap1(0))      s5: vB = min(lane2(u), swap5(2d))
    # s2: vA = min(u, swap2(d))     s6: pB = vB * m
    # s3: pA = vA * m               s7: accumB += pB       (out_a path)
    SWAP = AluInp.CURR_SWAP_OUT
    u_node = Bin(AluOp.MAX, Src0, Zero)       # capture-lane marker nodes
    scan_node = Bin(AluOp.ADD, Src0, Src1)
    pipeline = [
        _Stage(AluOp.SUBTRACT, Src0, SWAP),
        _Stage(AluOp.MAX, PREV, SWAP),
        _Stage(AluOp.MIN, PREV, SWAP),
        _Stage(AluOp.MULTIPLY, PREV, Src1),
        _Stage(AluOp.ADD, AluInp.CURR_ALU_OUT, PREV),
        _Stage(AluOp.MIN, AluInp.PREV_DELAY_2, SWAP),
        _Stage(AluOp.MULTIPLY, PREV, Src1),
        _Stage(AluOp.ADD, AluInp.CURR_ALU_OUT, PREV),
    ]
    p = _Placement(
        pipeline=pipeline, node_stage={},
        lane={Src0: 0, Src1: 1, u_node: 2, scan_node: 3},
        out_sel=OutSel.DELAY_3, accum_stage=7, captures=[(2, 2), (5, 3)],
    )
    latch_p = _Placement(
        pipeline=[_Stage(AluOp.BYPASS, PREV)] * 8, node_stage={},
        lane={C0: 0, Zero: 1, C2: 2, C1: 3},
        out_sel=OutSel.ALU_OUT, accum_stage=None, captures=[],
    )
    latch_ov = {
        0: _Stage(AluOp.BYPASS, C0, C0, swap=True),
        1: _Stage(AluOp.BYPASS, Zero, Zero, swap=True),
        2: _Stage(AluOp.BYPASS, C2, C2, swap=True),
        4: _Stage(AluOp.BYPASS, Zero, Zero, swap=True),
        5: _Stage(AluOp.BYPASS, C1, C1, swap=True),
        7: _Stage(AluOp.BYPASS, Zero, Zero, swap=True),
    }
    uops = [
        _assemble(_State(placement=latch_p, consume=(False, False),
                         overrides=latch_ov, trigger=COUNT_ONCE, repeat=1,
                         next=(1, 0, 0), write_out=False)),
        _assemble(_State(placement=p, consume=(False, False),
                         overrides={4: _Stage(AluOp.BYPASS, SWAP),
                                    7: _Stage(AluOp.BYPASS, SWAP)},
                         trigger=COUNT_ONCE, repeat=1, next=(2, 0, 0),
                         write_out=False)),
        _assemble(_State(placement=p, consume=(True, True))),
    ]
    _OPS_REGISTERED["dblwin"] = _reg_hand(
        "DBLWIN_ANT",
        Spec(body=minn(maxx(Src0 - C0, Zero), C2) * Src1, accum=_op_add,
             reference=_dblwin_ref),
        uops)
    return _OPS_REGISTERED


# --------------------------------------------------------------------------
# kernel build
# --------------------------------------------------------------------------
def _build(b_core, smooth_w, wk_is_ones):
    import concourse.bacc as bacc
    import concourse.mybir as mybir
    from concourse.tile import TileContext
    from concourse.bass import broadcast_tensor_aps

    ops = _register_custom_ops()
    RSQRT_NR, ADDMAX = ops["rsqrt_nr"], ops["addmax"]
    MULSUM, DBLWIN = ops["mulsum"], ops["dblwin"]

    f32 = mybir.dt.float32
    Alu = mybir.AluOpType
    Act = mybir.ActivationFunctionType

    n_tiles = b_core // P
    assert b_core % P == 0
    w0, w1, w2 = (float(x) for x in smooth_w)

    nc = bacc.Bacc(None, target_bir_lowering=False, debug=False)
    patch_in = nc.dram_tensor("patch", [b_core, HW], f32, kind="ExternalInput")
    # consts: iota36 repeated n_tiles times, then (iota36 - 64) repeated
    consts_in = nc.dram_tensor("consts", [P, 2 * n_tiles * NBINS], f32,
                               kind="ExternalInput")
    wk_in = None
    if not wk_is_ones:
        wk_in = nc.dram_tensor("wk", [P, HW], f32, kind="ExternalInput")
    out_t = nc.dram_tensor("angle", [b_core], f32, kind="ExternalOutput")

    with TileContext(nc) as tc:
        with tc.tile_pool(name="pool", bufs=2) as pool, \
             tc.tile_pool(name="persist", bufs=1) as pp:
            IOTA = pp.tile([P, n_tiles, NBINS], f32)
            IOTA64 = pp.tile([P, n_tiles, NBINS], f32)
            nc.sync.dma_start(IOTA[:], consts_in[:, 0:n_tiles * NBINS])
            nc.sync.dma_start(IOTA64[:], consts_in[:, n_tiles * NBINS:])
            WK = None
            if wk_in is not None:
                WK = pp.tile([P, HW], f32)
                nc.sync.dma_start(WK[:], wk_in[:])

            WS = pp.tile([P, n_tiles, 18], f32)   # W[2i]  (single window)
            WA = pp.tile([P, n_tiles, 18], f32)   # W2[2i] (double window)
            SMM = pp.tile([P, n_tiles, 1], f32)   # sum(m) per (patch, tile)
            EPS = pp.tile([P, 1], f32)            # sqrt bias (eps under root)
            nc.vector.memset(EPS[:], 6.4e-17)
            HEXT = pp.tile([P, n_tiles, NBINS + 2], f32)
            ANG = pp.tile([P, n_tiles], f32)

            n_groups = (n_tiles + GROUP - 1) // GROUP
            for g in range(n_groups):
                tiles = range(g * GROUP, min((g + 1) * GROUP, n_tiles))
                slot = {}
                # ---- phase A: sobel, magnitude (sqrt table set) ----
                for t in tiles:
                    s = t % GROUP
                    X = pool.tile([P, HW], f32, tag="x", bufs=3, name=f"x{t}")
                    nc.sync.dma_start(X[:], patch_in[t * P:(t + 1) * P, :])
                    X3 = X.rearrange("p (r c) -> p r c", c=PATCH)

                    SV = pool.tile([P, HW], f32, tag="sv", name=f"sv{t}")
                    # vertical [1,2,1] with replicate rows
                    nc.vector.scalar_tensor_tensor(
                        out=SV[:, 32:992], in0=X[:, 32:992], scalar=2.0,
                        in1=X[:, 0:960], op0=Alu.mult, op1=Alu.add)
                    nc.vector.tensor_tensor(
                        SV[:, 32:992], SV[:, 32:992], X[:, 64:1024], Alu.add)
                    nc.vector.scalar_tensor_tensor(
                        out=SV[:, 0:32], in0=X[:, 0:32], scalar=3.0,
                        in1=X[:, 32:64], op0=Alu.mult, op1=Alu.add)
                    nc.vector.scalar_tensor_tensor(
                        out=SV[:, 992:1024], in0=X[:, 992:1024], scalar=3.0,
                        in1=X[:, 960:992], op0=Alu.mult, op1=Alu.add)
                    SV3 = SV.rearrange("p (r c) -> p r c", c=PATCH)

                    GX = pool.tile([P, HW], f32, tag=f"gx{s}", bufs=1,
                                   name=f"gx{t}")
                    GX3 = GX.rearrange("p (r c) -> p r c", c=PATCH)
                    # horizontal central difference: contiguous flat pass
                    # (wrong only at the c=0/c=31 columns), then strided
                    # column fixups.  Strided [32,30] DVE ops run ~3x slower
                    # than contiguous; [32,1] fixups are cheap.
                    nc.vector.tensor_tensor(
                        GX[:, 1:1023], SV[:, 2:1024], SV[:, 0:1022],
                        Alu.subtract)
                    nc.vector.tensor_tensor(
                        GX3[:, :, 0:1], SV3[:, :, 1:2], SV3[:, :, 0:1],
                        Alu.subtract)
                    nc.vector.tensor_tensor(
                        GX3[:, :, 31:32], SV3[:, :, 31:32], SV3[:, :, 30:31],
                        Alu.subtract)

                    SH = pool.tile([P, HW], f32, tag="sh", name=f"sh{t}")
                    SH3 = SH.rearrange("p (r c) -> p r c", c=PATCH)
                    # horizontal [1,2,1]: flat contiguous + column fixups
                    nc.vector.scalar_tensor_tensor(
                        out=SH[:, 1:1023], in0=X[:, 1:1023], scalar=2.0,
                        in1=X[:, 0:1022], op0=Alu.mult, op1=Alu.add)
                    nc.vector.tensor_tensor(
                        SH[:, 1:1023], SH[:, 1:1023], X[:, 2:1024],
                        Alu.add)
                    nc.vector.scalar_tensor_tensor(
                        out=SH3[:, :, 0:1], in0=X3[:, :, 0:1], scalar=3.0,
                        in1=X3[:, :, 1:2], op0=Alu.mult, op1=Alu.add)
                    nc.vector.scalar_tensor_tensor(
                        out=SH3[:, :, 31:32], in0=X3[:, :, 31:32], scalar=3.0,
                        in1=X3[:, :, 30:31], op0=Alu.mult, op1=Alu.add)

                    GY = pool.tile([P, HW], f32, tag=f"gy{s}", bufs=1,
                                   name=f"gy{t}")
                    # vertical central difference with replicate rows
                    nc.vector.tensor_tensor(
                        GY[:, 32:992], SH[:, 64:1024], SH[:, 0:960],
                        Alu.subtract)
                    nc.vector.tensor_tensor(
                        GY[:, 0:32], SH[:, 32:64], SH[:, 0:32], Alu.subtract)
                    nc.vector.tensor_tensor(
                        GY[:, 992:1024], SH[:, 992:1024], SH[:, 960:992],
                        Alu.subtract)

                    if WK is not None:
                        nc.vector.tensor_tensor(GX[:], GX[:], WK[:], Alu.mult)
                        nc.vector.tensor_tensor(GY[:], GY[:], WK[:], Alu.mult)

                    # g2 = gx^2 + gy^2 + eps  (eps scaled by 8^2 vs reference)
                    # sv/sh slots are dead here; reuse their tags for squares.
                    # Exact fp32 multiplies on GPSIMD (ACT Square is ~1e-5
                    # off, which poisons the magnitude beyond repair).
                    X2 = pool.tile([P, HW], f32, tag="sv", name=f"x2{t}")
                    Y2 = pool.tile([P, HW], f32, tag="sh", name=f"y2{t}")
                    nc.gpsimd.tensor_tensor(X2[:], GX[:], GX[:], Alu.mult)
                    nc.gpsimd.tensor_tensor(Y2[:], GY[:], GY[:], Alu.mult)
                    G2 = pool.tile([P, HW], f32, tag="g2", name=f"g2{t}")
                    nc.vector.tensor_tensor(G2[:], X2[:], Y2[:], Alu.add)
                    M = pool.tile([P, HW], f32, tag=f"m{s}", bufs=1,
                                   name=f"m{t}")
                    # eps folded into the ACT free affine: sqrt(g2 + eps)
                    nc.scalar.activation(M[:], G2[:], Act.Sqrt, bias=EPS[:])
                    # one Newton step: m = g2 * nr(1/sqrt); sum(m) fused out
                    RC = pool.tile([P, HW], f32, tag="rc", name=f"rc{t}")
                    SC = pool.tile([P, HW], f32, tag="sc", name=f"sc{t}")
                    nc.vector.reciprocal_approx_fast(RC[:], M[:])
                    nc.vector._custom_dve(RSQRT_NR, out=SC[:], in0=G2[:],
                                          in1=RC[:], s0=1.5, s1=0.5)
                    nc.vector._custom_dve(MULSUM, out=M[:], in0=G2[:],
                                          in1=SC[:],
                                          accum_out=SMM[:, t, 0:1])
                    slot[t] = (GX, GY, M)

                # ---- phase B: orientation + histogram (sigmoid table set) --
                for t in tiles:
                    GX, GY, M = slot[t]
                    # d = max(m + gx, 1e-30): the clamp both avoids the
                    # recip(0)=NaN edge and pins rounding-negative d to the
                    # correct wrap side.
                    D = pool.tile([P, HW], f32, tag="g2", name=f"d{t}")
                    nc.vector._custom_dve(ADDMAX, out=D[:], in0=M[:],
                                          in1=GX[:], s0=1e-30)
                    RC = pool.tile([P, HW], f32, tag="rc", name=f"rcb{t}")
                    SC = pool.tile([P, HW], f32, tag="sc", name=f"scb{t}")
                    nc.vector.reciprocal_approx_fast(RC[:], D[:])
                    nc.vector.tensor_tensor(SC[:], GY[:], RC[:], Alu.mult)
                    A = pool.tile([P, HW], f32, tag="a", name=f"a{t}")
                    nc.scalar.activation(A[:], SC[:], Act.Arctan)

                    # 18 dual-window passes: W[2i] via stride-0 scan out,
                    # W2[2i] via accumulator.
                    for i in range(18):
                        b = (2 * i - 18) * PI / 36.0
                        _, cell = broadcast_tensor_aps(A[:],
                                                       WS[:, t, i:i + 1])
                        nc.vector._custom_dve(
                            DBLWIN, out=cell, in0=A[:], in1=M[:],
                            s0=float(b), s1=float(2 * DELTA),
                            imm2=float(DELTA),
                            accum_out=WA[:, t, i:i + 1])

            # ---- tail: assemble hist, smoothing, argmax, refine (batched) --
            # odd bins 1,3..35 -> HEXT cols 2,4..36:  2*WS - WA
            nc.vector.scalar_tensor_tensor(
                out=HEXT[:, :, 2:38:2], in0=WS[:], scalar=2.0, in1=WA[:],
                op0=Alu.mult, op1=Alu.subtract)
            # WAS = WA - WS (kept in WA; WA dead after)
            nc.vector.tensor_tensor(WA[:], WA[:], WS[:], Alu.subtract)
            # even bins 2,4..34 -> HEXT cols 3,5..35: WAS[i] - WS[i+1], i=0..16
            nc.vector.tensor_tensor(
                HEXT[:, :, 3:37:2], WA[:, :, 0:17], WS[:, :, 1:18],
                Alu.subtract)
            # bin 0 -> col 1: delta*sum(m) - WS[0] + WAS[17]
            nc.vector.scalar_tensor_tensor(
                out=HEXT[:, :, 1:2], in0=SMM[:], scalar=float(DELTA),
                in1=WS[:, :, 0:1], op0=Alu.mult, op1=Alu.subtract)
            nc.vector.tensor_tensor(
                HEXT[:, :, 1:2], HEXT[:, :, 1:2], WA[:, :, 17:18], Alu.add)
            # wrap columns
            nc.vector.tensor_copy(HEXT[:, :, 0:1], HEXT[:, :, 36:37])
            nc.vector.tensor_copy(HEXT[:, :, 37:38], HEXT[:, :, 1:2])

            SM = pp.tile([P, n_tiles, NBINS], f32)
            nc.vector.tensor_scalar(SM[:], HEXT[:, :, 2:38], w2, None,
                                    Alu.mult)
            nc.vector.scalar_tensor_tensor(
                out=SM[:], in0=HEXT[:, :, 0:36], scalar=w0, in1=SM[:],
                op0=Alu.mult, op1=Alu.add)
            HS = pp.tile([P, n_tiles, NBINS], f32)
            nc.vector.scalar_tensor_tensor(
                out=HS[:], in0=HEXT[:, :, 1:37], scalar=w1, in1=SM[:],
                op0=Alu.mult, op1=Alu.add)

            VMAX = pp.tile([P, n_tiles, 1], f32)
            nc.vector.tensor_reduce(VMAX[:], HS[:], mybir.AxisListType.X,
                                    Alu.max)
            EQ = pp.tile([P, n_tiles, NBINS], f32)
            hs_b, vmax_b = broadcast_tensor_aps(HS[:], VMAX[:])
            nc.vector.tensor_tensor(EQ[:], hs_b, vmax_b, Alu.is_equal)
            nc.vector.tensor_tensor(EQ[:], EQ[:], IOTA64[:], Alu.mult)
            IDX = pp.tile([P, n_tiles, 1], f32)
            nc.vector.tensor_reduce(IDX[:], EQ[:], mybir.AxisListType.X,
                                    Alu.min)
            nc.vector.tensor_scalar(IDX[:], IDX[:], 64.0, None, Alu.add)

            def neighbor_value(shift, wrap_thr, wrap_add, nm):
                IDXN = pp.tile([P, n_tiles, 1], f32, name=f"idxn_{nm}")
                nc.vector.tensor_scalar(IDXN[:], IDX[:], float(shift), None,
                                        Alu.add)
                WADJ = pp.tile([P, n_tiles, 1], f32, name=f"wadj_{nm}")
                if wrap_add < 0:
                    nc.vector.tensor_scalar(WADJ[:], IDXN[:], wrap_thr,
                                            float(wrap_add), Alu.is_gt,
                                            Alu.mult)
                else:
                    nc.vector.tensor_scalar(WADJ[:], IDXN[:], wrap_thr,
                                            float(wrap_add), Alu.is_lt,
                                            Alu.mult)
                nc.vector.tensor_tensor(IDXN[:], IDXN[:], WADJ[:], Alu.add)
                DIF = pp.tile([P, n_tiles, NBINS], f32, name=f"dif_{nm}")
                iota_b, idxn_b = broadcast_tensor_aps(IOTA[:], IDXN[:])
                nc.vector.tensor_tensor(DIF[:], iota_b, idxn_b, Alu.subtract)
                nc.vector.tensor_scalar(DIF[:], DIF[:], 0.0, None,
                                        Alu.is_equal)
                nc.vector.tensor_tensor(DIF[:], DIF[:], HS[:], Alu.mult)
                V = pp.tile([P, n_tiles, 1], f32, name=f"v_{nm}")
                nc.vector.tensor_reduce(V[:], DIF[:], mybir.AxisListType.X,
                                        Alu.add)
                return V

            VP = neighbor_value(+1, 35.5, -36.0, "p")
            VM = neighbor_value(-1, -0.5, +36.0, "m")

            NUM = pp.tile([P, n_tiles, 1], f32)
            nc.vector.tensor_tensor(NUM[:], VP[:], VM[:], Alu.subtract)
            SUMN = pp.tile([P, n_tiles, 1], f32)
            nc.vector.tensor_tensor(SUMN[:], VP[:], VM[:], Alu.add)
            DEN = pp.tile([P, n_tiles, 1], f32)
            nc.vector.tensor_scalar(DEN[:], VMAX[:], 2.0, None, Alu.mult)
            nc.vector.tensor_tensor(DEN[:], DEN[:], SUMN[:], Alu.subtract)
            RECD = pp.tile([P, n_tiles, 1], f32)
            SCD = pp.tile([P, n_tiles, 1], f32)
            nc.vector.reciprocal_approx_accurate(RECD[:], DEN[:], SCD[:])
            REF = pp.tile([P, n_tiles, 1], f32)
            nc.vector.scalar_tensor_tensor(
                out=REF[:], in0=NUM[:], scalar=0.5, in1=RECD[:],
                op0=Alu.mult, op1=Alu.mult)
            nc.vector.tensor_tensor(REF[:], IDX[:], REF[:], Alu.add)
            nc.vector.tensor_scalar(ANG[:], REF[:, :, 0], -2.0 * PI / NBINS,
                                    PI, Alu.mult, Alu.add)

            out_view = out_t[:].rearrange("(t p) -> p t", p=P)
            nc.sync.dma_start(out_view, ANG[:])

    nc.compile()
    return nc


def _get_built(b_core, smooth_w, wk_is_ones):
    key = (b_core, tuple(float(x) for x in smooth_w), bool(wk_is_ones))
    if key not in _BUILD_CACHE:
        _BUILD_CACHE[key] = _build(b_core, smooth_w, wk_is_ones)
    return _BUILD_CACHE[key]


# --------------------------------------------------------------------------
# host entry point
# --------------------------------------------------------------------------
def kernel(patch, weight_kernel, smooth_w):
    from concourse import bass_utils

    patch = np.ascontiguousarray(np.asarray(patch, dtype=np.float32))
    weight_kernel = np.asarray(weight_kernel, dtype=np.float32)
    smooth_w = np.asarray(smooth_w, dtype=np.float32)

    B = patch.shape[0]
    assert B % (N_CORES * P) == 0, f"B={B} not divisible by {N_CORES * P}"
    b_core = B // N_CORES
    n_tiles = b_core // P

    wk_is_ones = bool(np.all(weight_kernel == 1.0))
    nc = _get_built(b_core, smooth_w, wk_is_ones)

    x = patch.reshape(N_CORES, b_core, HW)

    iota = np.tile(np.arange(NBINS, dtype=np.float32), n_tiles)
    consts_row = np.concatenate([iota, iota - 64.0]).astype(np.float32)
    consts = np.ascontiguousarray(
        np.broadcast_to(consts_row, (P, consts_row.size)))

    in_maps = []
    for i in range(N_CORES):
        m = {"patch": np.ascontiguousarray(x[i]), "consts": consts}
        if not wk_is_ones:
            m["wk"] = np.ascontiguousarray(
                np.broadcast_to(weight_kernel.reshape(-1), (P, HW)))
        in_maps.append(m)

    res = bass_utils.run_bass_kernel_spmd(nc, in_maps,
                                          core_ids=list(range(N_CORES)))
    out = np.concatenate([r["angle"] for r in res.results])
    return out.astype(np.float32)


# revision 23
# speedup vs baseline: 1.1901x; 1.1104x over previous
"""Trainium2 Bass kernel for CustomizablePatchDominantGradientOrientation.

Pipeline per patch (32x32, fp32):
  sobel (replicate pad, [1,2,1]x[-1,0,1] separable; /8 dropped - the final
  angle is invariant to a global scale on (gx, gy, mag))
  mag = sqrt(gx^2+gy^2+eps'), theta = 2*atan(gy/(mag+gx))  (half-angle atan2)
  36-bin soft histogram via 18 dual-window custom-DVE passes: pass i
  (boundary b = (2i-18)*pi/36) emits in ONE instruction both
    W[2i]  = sum m*clamp(a-b, 0, d)    (in-pipe prefix-sum, streamed to a
                                        stride-0 out AP; final write = total)
    W2[2i] = sum m*clamp(a-b, 0, 2d)   (accumulator out_a path)
  with W[2i+1] = W2[2i] - W[2i]; hist[k] = W[k-1] - W[k] (bounded masks ->
  accumulation roundoff stays tiny).  Then circular [w0,w1,w2] smoothing,
  argmax, parabolic refinement -> angle.

Data parallel: B=32768 patches sharded over 8 NeuronCores (4096 each);
per core 32 tiles of [128 patches x 1024 pixels].  Layout is patch-major:
partitions = patches, free axis = pixels.
"""

import math

import numpy as np

NBINS = 36
PI = math.pi
PATCH = 32
HW = PATCH * PATCH
P = 128          # partitions (patches per tile)
N_CORES = 8
GROUP = 4        # tiles per ACT-table-set phase group
DELTA = PI / 36.0  # window width in atan units (theta = 2a, bin = 2pi/36)

_BUILD_CACHE = {}
_OPS_REGISTERED = {}


# --------------------------------------------------------------------------
# custom DVE ops
# --------------------------------------------------------------------------
def _register_custom_ops():
    """Register the fused ops at runtime (row assignment + sha pin, exactly
    what a source-level `OPS.append` would do).  DBLWIN_ANT's uop program is
    hand-assembled (dual outputs: in-pipe scan -> out port, accum -> out_a);
    its compiled DveOpSpec is pre-seeded into the compile cache."""
    if _OPS_REGISTERED:
        return _OPS_REGISTERED
    from operator import add as _op_add

    import concourse.dve_ops as dve_ops
    from concourse.dve_ops import DveOp, _COMPILE_CACHE
    from concourse.dve_spec import (
        Spec, Src0, Src1, C0, C1, C2, Zero, relu, minn, maxx, lower,
        _has_src1, Bin, _Placement, _State, _Stage, _assemble, COUNT_ONCE,
        PREV, sq as _sq,
    )
    from concourse.dve_uop import AluInp, AluOp, DveOpSpec, OutSel
    from concourse.dve_uop import DveOpSpec

    def _reg(name, spec):
        if name in dve_ops._SUB_OPCODE_FOR_NAME:
            for op in dve_ops.OPS:
                if op.name == name:
                    return op
        row = dve_ops._CUSTOM_DVE_ROW_BASE + len(dve_ops.OPS)
        assert row < 0x20, "custom-DVE row budget exhausted"
        dve_ops._SUB_OPCODE_FOR_NAME[name] = row
        shas = {}
        for ver in ("v3", "v4"):
            s = DveOpSpec(name=name, opcode=row, uops=lower(spec, ver=ver),
                          rd1_en=_has_src1(spec))
            shas[ver] = s.sha(ver)
        op = DveOp(name, spec, subdim=False, uops_sha=shas)
        dve_ops.OPS.append(op)
        dve_ops.CUSTOM_DVE_SPECS[name] = spec
        return op

    def _reg_hand(name, spec, uops):
        """Register with a hand-assembled v3 uop program (bypasses lower())."""
        if name in dve_ops._SUB_OPCODE_FOR_NAME:
            for op in dve_ops.OPS:
                if op.name == name:
                    return op
        row = dve_ops._CUSTOM_DVE_ROW_BASE + len(dve_ops.OPS)
        assert row < 0x20, "custom-DVE row budget exhausted"
        dve_ops._SUB_OPCODE_FOR_NAME[name] = row
        s3 = DveOpSpec(name=name, opcode=row, uops=uops, rd1_en=True)
        op = DveOp(name, spec, subdim=False, uops_sha={"v3": s3.sha("v3")})
        dve_ops.OPS.append(op)
        dve_ops.CUSTOM_DVE_SPECS[name] = spec
        _COMPILE_CACHE[(name, "v3")] = s3
        return op

    def _addmax_ref(in0, in1, s0, s1, imm2):
        return np.maximum(in0 + in1, s0).astype(np.float32)

    def _sqsum_ref(in0, in1, s0, s1, imm2):
        return (in0 * in0 + in1 * in1 + s0).astype(np.float32)

    def _mag_nr_sum_ref(in0, in1, s0, s1, imm2):
        o = (in0 * ((s0 - in0 * in1 * in1 * s1) * in1)).astype(np.float32)
        return o, o.reshape(o.shape[0], -1).sum(axis=-1, keepdims=True)

    def _dblwin_ref(in0, in1, s0, s1, imm2):
        x = (in0 - np.float32(s0)).astype(np.float32)
        u = np.maximum(x, np.float32(0))
        pA = (np.minimum(u, np.float32(imm2)) * in1).astype(np.float32)
        pB = (np.minimum(u, np.float32(s1)) * in1).astype(np.float32)
        out = np.cumsum(pA.astype(np.float64), axis=-1).astype(np.float32)
        acc = (pB.reshape(pB.shape[0], -1).astype(np.float64)
               .sum(axis=-1, keepdims=True).astype(np.float32))
        return out, acc

    _OPS_REGISTERED["addmax"] = _reg(
        "ADD_MAX_ANT",
        Spec(body=maxx(Src0 + Src1, C0), reference=_addmax_ref))
    # g2 = gx^2 + gy^2 + eps
    _OPS_REGISTERED["sqsum"] = _reg(
        "SQ_SUM_ANT",
        Spec(body=_sq(Src0) + _sq(Src1) + C0, reference=_sqsum_ref))
    # m = g2*(c0 - g2*rc^2*c1)*rc (Newton step on the rsqrt seed rc),
    # with sum(m) on the accumulator.
    _OPS_REGISTERED["magnr"] = _reg(
        "MAG_NR_SUM_ANT",
        Spec(body=Src0 * ((C0 - Src0 * _sq(Src1) * C1) * Src1),
             accum=_op_add, reference=_mag_nr_sum_ref))

    # -- DBLWIN_ANT: hand-assembled dual-window clamp-mask reduction --------
    # s0: x = a - swap0(b)          s4: scanA += pA        (lane3 -> out port)
    # s1: u = max(x, swap1(0))      s5: vB = min(lane2(u), swap5(2d))
    # s2: vA = min(u, swap2(d))     s6: pB = vB * m
    # s3: pA = vA * m               s7: accumB += pB       (out_a path)
    SWAP = AluInp.CURR_SWAP_OUT
    u_node = Bin(AluOp.MAX, Src0, Zero)       # capture-lane marker nodes
    scan_node = Bin(AluOp.ADD, Src0, Src1)
    pipeline = [
        _Stage(AluOp.SUBTRACT, Src0, SWAP),
        _Stage(AluOp.MAX, PREV, SWAP),
        _Stage(AluOp.MIN, PREV, SWAP),
        _Stage(AluOp.MULTIPLY, PREV, Src1),
        _Stage(AluOp.ADD, AluInp.CURR_ALU_OUT, PREV),
        _Stage(AluOp.MIN, AluInp.PREV_DELAY_2, SWAP),
        _Stage(AluOp.MULTIPLY, PREV, Src1),
        _Stage(AluOp.ADD, AluInp.CURR_ALU_OUT, PREV),
    ]
    p = _Placement(
        pipeline=pipeline, node_stage={},
        lane={Src0: 0, Src1: 1, u_node: 2, scan_node: 3},
        out_sel=OutSel.DELAY_3, accum_stage=7, captures=[(2, 2), (5, 3)],
    )
    latch_p = _Placement(
        pipeline=[_Stage(AluOp.BYPASS, PREV)] * 8, node_stage={},
        lane={C0: 0, Zero: 1, C2: 2, C1: 3},
        out_sel=OutSel.ALU_OUT, accum_stage=None, captures=[],
    )
    latch_ov = {
        0: _Stage(AluOp.BYPASS, C0, C0, swap=True),
        1: _Stage(AluOp.BYPASS, Zero, Zero, swap=True),
        2: _Stage(AluOp.BYPASS, C2, C2, swap=True),
        4: _Stage(AluOp.BYPASS, Zero, Zero, swap=True),
        5: _Stage(AluOp.BYPASS, C1, C1, swap=True),
        7: _Stage(AluOp.BYPASS, Zero, Zero, swap=True),
    }
    uops = [
        _assemble(_State(placement=latch_p, consume=(False, False),
                         overrides=latch_ov, trigger=COUNT_ONCE, repeat=1,
                         next=(1, 0, 0), write_out=False)),
        _assemble(_State(placement=p, consume=(False, False),
                         overrides={4: _Stage(AluOp.BYPASS, SWAP),
                                    7: _Stage(AluOp.BYPASS, SWAP)},
                         trigger=COUNT_ONCE, repeat=1, next=(2, 0, 0),
                         write_out=False)),
        _assemble(_State(placement=p, consume=(True, True))),
    ]
    _OPS_REGISTERED["dblwin"] = _reg_hand(
        "DBLWIN_ANT",
        Spec(body=minn(maxx(Src0 - C0, Zero), C2) * Src1, accum=_op_add,
             reference=_dblwin_ref),
        uops)
    return _OPS_REGISTERED


# --------------------------------------------------------------------------
# kernel build
# --------------------------------------------------------------------------
def _build(b_core, smooth_w, wk_is_ones):
    import concourse.bacc as bacc
    import concourse.mybir as mybir
    from concourse.tile import TileContext
    from concourse.bass import broadcast_tensor_aps

    ops = _register_custom_ops()
    ADDMAX, SQSUM = ops["addmax"], ops["sqsum"]
    MAGNR, DBLWIN = ops["magnr"], ops["dblwin"]

    f32 = mybir.dt.float32
    Alu = mybir.AluOpType
    Act = mybir.ActivationFunctionType

    n_tiles = b_core // P
    assert b_core % P == 0
    w0, w1, w2 = (float(x) for x in smooth_w)

    nc = bacc.Bacc(None, target_bir_lowering=False, debug=False)
    patch_in = nc.dram_tensor("patch", [b_core, HW], f32, kind="ExternalInput")
    # consts: iota36 repeated n_tiles times, then (iota36 - 64) repeated
    consts_in = nc.dram_tensor("consts", [P, 2 * n_tiles * NBINS], f32,
                               kind="ExternalInput")
    wk_in = None
    if not wk_is_ones:
        wk_in = nc.dram_tensor("wk", [P, HW], f32, kind="ExternalInput")
    out_t = nc.dram_tensor("angle", [b_core], f32, kind="ExternalOutput")

    with TileContext(nc) as tc:
        with tc.tile_pool(name="pool", bufs=2) as pool, \
             tc.tile_pool(name="persist", bufs=1) as pp:
            IOTA = pp.tile([P, n_tiles, NBINS], f32)
            IOTA64 = pp.tile([P, n_tiles, NBINS], f32)
            nc.sync.dma_start(IOTA[:], consts_in[:, 0:n_tiles * NBINS])
            nc.sync.dma_start(IOTA64[:], consts_in[:, n_tiles * NBINS:])
            WK = None
            if wk_in is not None:
                WK = pp.tile([P, HW], f32)
                nc.sync.dma_start(WK[:], wk_in[:])

            WS = pp.tile([P, n_tiles, 18], f32)   # W[2i]  (single window)
            WA = pp.tile([P, n_tiles, 18], f32)   # W2[2i] (double window)
            SMM = pp.tile([P, n_tiles, 1], f32)   # sum(m) per (patch, tile)
            HEXT = pp.tile([P, n_tiles, NBINS + 2], f32)
            ANG = pp.tile([P, n_tiles], f32)

            n_groups = (n_tiles + GROUP - 1) // GROUP
            for g in range(n_groups):
                tiles = range(g * GROUP, min((g + 1) * GROUP, n_tiles))
                slot = {}
                # ---- phase A: sobel, magnitude (sqrt table set) ----
                for t in tiles:
                    s = t % GROUP
                    X = pool.tile([P, HW], f32, tag="x", bufs=3, name=f"x{t}")
                    nc.sync.dma_start(X[:], patch_in[t * P:(t + 1) * P, :])
                    X3 = X.rearrange("p (r c) -> p r c", c=PATCH)

                    SV = pool.tile([P, HW], f32, tag="sv", name=f"sv{t}")
                    # vertical [1,2,1] with replicate rows
                    nc.vector.scalar_tensor_tensor(
                        out=SV[:, 32:992], in0=X[:, 32:992], scalar=2.0,
                        in1=X[:, 0:960], op0=Alu.mult, op1=Alu.add)
                    nc.vector.tensor_tensor(
                        SV[:, 32:992], SV[:, 32:992], X[:, 64:1024], Alu.add)
                    nc.vector.scalar_tensor_tensor(
                        out=SV[:, 0:32], in0=X[:, 0:32], scalar=3.0,
                        in1=X[:, 32:64], op0=Alu.mult, op1=Alu.add)
                    nc.vector.scalar_tensor_tensor(
                        out=SV[:, 992:1024], in0=X[:, 992:1024], scalar=3.0,
                        in1=X[:, 960:992], op0=Alu.mult, op1=Alu.add)
                    SV3 = SV.rearrange("p (r c) -> p r c", c=PATCH)

                    GX = pool.tile([P, HW], f32, tag=f"gx{s}", bufs=1,
                                   name=f"gx{t}")
                    GX3 = GX.rearrange("p (r c) -> p r c", c=PATCH)
                    # horizontal central difference: contiguous flat pass
                    # (wrong only at the c=0/c=31 columns), then strided
                    # column fixups.  Strided [32,30] DVE ops run ~3x slower
                    # than contiguous; [32,1] fixups are cheap.
                    nc.vector.tensor_tensor(
                        GX[:, 1:1023], SV[:, 2:1024], SV[:, 0:1022],
                        Alu.subtract)
                    nc.vector.tensor_tensor(
                        GX3[:, :, 0:1], SV3[:, :, 1:2], SV3[:, :, 0:1],
                        Alu.subtract)
                    nc.vector.tensor_tensor(
                        GX3[:, :, 31:32], SV3[:, :, 31:32], SV3[:, :, 30:31],
                        Alu.subtract)

                    SH = pool.tile([P, HW], f32, tag="sh", name=f"sh{t}")
                    SH3 = SH.rearrange("p (r c) -> p r c", c=PATCH)
                    # horizontal [1,2,1]: flat contiguous + column fixups
                    nc.vector.scalar_tensor_tensor(
                        out=SH[:, 1:1023], in0=X[:, 1:1023], scalar=2.0,
                        in1=X[:, 0:1022], op0=Alu.mult, op1=Alu.add)
                    nc.vector.tensor_tensor(
                        SH[:, 1:1023], SH[:, 1:1023], X[:, 2:1024],
                        Alu.add)
                    nc.vector.scalar_tensor_tensor(
                        out=SH3[:, :, 0:1], in0=X3[:, :, 0:1], scalar=3.0,
                        in1=X3[:, :, 1:2], op0=Alu.mult, op1=Alu.add)
                    nc.vector.scalar_tensor_tensor(
                        out=SH3[:, :, 31:32], in0=X3[:, :, 31:32], scalar=3.0,
                        in1=X3[:, :, 30:31], op0=Alu.mult, op1=Alu.add)

                    GY = pool.tile([P, HW], f32, tag=f"gy{s}", bufs=1,
                                   name=f"gy{t}")
                    # vertical central difference with replicate rows
                    nc.vector.tensor_tensor(
                        GY[:, 32:992], SH[:, 64:1024], SH[:, 0:960],
                        Alu.subtract)
                    nc.vector.tensor_tensor(
                        GY[:, 0:32], SH[:, 32:64], SH[:, 0:32], Alu.subtract)
                    nc.vector.tensor_tensor(
                        GY[:, 992:1024], SH[:, 992:1024], SH[:, 960:992],
                        Alu.subtract)

                    if WK is not None:
                        nc.vector.tensor_tensor(GX[:], GX[:], WK[:], Alu.mult)
                        nc.vector.tensor_tensor(GY[:], GY[:], WK[:], Alu.mult)

                    # g2 = gx^2 + gy^2 + eps  (eps scaled by 8^2 vs
                    # reference) - exact DVE fp32 multiplies, one fused op.
                    G2 = pool.tile([P, HW], f32, tag="g2", name=f"g2{t}")
                    nc.vector._custom_dve(SQSUM, out=G2[:], in0=GX[:],
                                          in1=GY[:], s0=6.4e-17)
                    M = pool.tile([P, HW], f32, tag=f"m{s}", bufs=1,
                                   name=f"m{t}")
                    nc.scalar.activation(M[:], G2[:], Act.Sqrt)
                    # one Newton step + final m = g2*nr; sum(m) fused out
                    RC = pool.tile([P, HW], f32, tag="rc", name=f"rc{t}")
                    nc.vector.reciprocal_approx_fast(RC[:], M[:])
                    nc.vector._custom_dve(MAGNR, out=M[:], in0=G2[:],
                                          in1=RC[:], s0=1.5, s1=0.5,
                                          accum_out=SMM[:, t, 0:1])
                    slot[t] = (GX, GY, M)

                # ---- phase B: orientation + histogram (sigmoid table set) --
                for t in tiles:
                    GX, GY, M = slot[t]
                    # d = max(m + gx, 1e-30): the clamp both avoids the
                    # recip(0)=NaN edge and pins rounding-negative d to the
                    # correct wrap side.
                    D = pool.tile([P, HW], f32, tag="g2", name=f"d{t}")
                    nc.vector._custom_dve(ADDMAX, out=D[:], in0=M[:],
                                          in1=GX[:], s0=1e-30)
                    RC = pool.tile([P, HW], f32, tag="rc", name=f"rcb{t}")
                    SC = pool.tile([P, HW], f32, tag="sc", name=f"scb{t}")
                    nc.vector.reciprocal_approx_fast(RC[:], D[:])
                    nc.vector.tensor_tensor(SC[:], GY[:], RC[:], Alu.mult)
                    A = pool.tile([P, HW], f32, tag="a", name=f"a{t}")
                    nc.scalar.activation(A[:], SC[:], Act.Arctan)

                    # 18 dual-window passes: W[2i] via stride-0 scan out,
                    # W2[2i] via accumulator.
                    for i in range(18):
                        b = (2 * i - 18) * PI / 36.0
                        _, cell = broadcast_tensor_aps(A[:],
                                                       WS[:, t, i:i + 1])
                        nc.vector._custom_dve(
                            DBLWIN, out=cell, in0=A[:], in1=M[:],
                            s0=float(b), s1=float(2 * DELTA),
                            imm2=float(DELTA),
                            accum_out=WA[:, t, i:i + 1])

            # ---- tail: assemble hist, smoothing, argmax, refine (batched) --
            # odd bins 1,3..35 -> HEXT cols 2,4..36:  2*WS - WA
            nc.vector.scalar_tensor_tensor(
                out=HEXT[:, :, 2:38:2], in0=WS[:], scalar=2.0, in1=WA[:],
                op0=Alu.mult, op1=Alu.subtract)
            # WAS = WA - WS (kept in WA; WA dead after)
            nc.vector.tensor_tensor(WA[:], WA[:], WS[:], Alu.subtract)
            # even bins 2,4..34 -> HEXT cols 3,5..35: WAS[i] - WS[i+1], i=0..16
            nc.vector.tensor_tensor(
                HEXT[:, :, 3:37:2], WA[:, :, 0:17], WS[:, :, 1:18],
                Alu.subtract)
            # bin 0 -> col 1: delta*sum(m) - WS[0] + WAS[17]
            nc.vector.scalar_tensor_tensor(
                out=HEXT[:, :, 1:2], in0=SMM[:], scalar=float(DELTA),
                in1=WS[:, :, 0:1], op0=Alu.mult, op1=Alu.subtract)
            nc.vector.tensor_tensor(
                HEXT[:, :, 1:2], HEXT[:, :, 1:2], WA[:, :, 17:18], Alu.add)
            # wrap columns
            nc.vector.tensor_copy(HEXT[:, :, 0:1], HEXT[:, :, 36:37])
            nc.vector.tensor_copy(HEXT[:, :, 37:38], HEXT[:, :, 1:2])

            SM = pp.tile([P, n_tiles, NBINS], f32)
            nc.vector.tensor_scalar(SM[:], HEXT[:, :, 2:38], w2, None,
                                    Alu.mult)
            nc.vector.scalar_tensor_tensor(
                out=SM[:], in0=HEXT[:, :, 0:36], scalar=w0, in1=SM[:],
                op0=Alu.mult, op1=Alu.add)
            HS = pp.tile([P, n_tiles, NBINS], f32)
            nc.vector.scalar_tensor_tensor(
                out=HS[:], in0=HEXT[:, :, 1:37], scalar=w1, in1=SM[:],
                op0=Alu.mult, op1=Alu.add)

            VMAX = pp.tile([P, n_tiles, 1], f32)
            nc.vector.tensor_reduce(VMAX[:], HS[:], mybir.AxisListType.X,
                                    Alu.max)
            EQ = pp.tile([P, n_tiles, NBINS], f32)
            hs_b, vmax_b = broadcast_tensor_aps(HS[:], VMAX[:])
            nc.vector.tensor_tensor(EQ[:], hs_b, vmax_b, Alu.is_equal)
            nc.vector.tensor_tensor(EQ[:], EQ[:], IOTA64[:], Alu.mult)
            IDX = pp.tile([P, n_tiles, 1], f32)
            nc.vector.tensor_reduce(IDX[:], EQ[:], mybir.AxisListType.X,
                                    Alu.min)
            nc.vector.tensor_scalar(IDX[:], IDX[:], 64.0, None, Alu.add)

            def neighbor_value(shift, wrap_thr, wrap_add, nm):
                IDXN = pp.tile([P, n_tiles, 1], f32, name=f"idxn_{nm}")
                nc.vector.tensor_scalar(IDXN[:], IDX[:], float(shift), None,
                                        Alu.add)
                WADJ = pp.tile([P, n_tiles, 1], f32, name=f"wadj_{nm}")
                if wrap_add < 0:
                    nc.vector.tensor_scalar(WADJ[:], IDXN[:], wrap_thr,
                                            float(wrap_add), Alu.is_gt,
                                            Alu.mult)
                else:
                    nc.vector.tensor_scalar(WADJ[:], IDXN[:], wrap_thr,
                                            float(wrap_add), Alu.is_lt,
                                            Alu.mult)
                nc.vector.tensor_tensor(IDXN[:], IDXN[:], WADJ[:], Alu.add)
                DIF = pp.tile([P, n_tiles, NBINS], f32, name=f"dif_{nm}")
                iota_b, idxn_b = broadcast_tensor_aps(IOTA[:], IDXN[:])
                nc.vector.tensor_tensor(DIF[:], iota_b, idxn_b, Alu.subtract)
                nc.vector.tensor_scalar(DIF[:], DIF[:], 0.0, None,
                                        Alu.is_equal)
                nc.vector.tensor_tensor(DIF[:], DIF[:], HS[:], Alu.mult)
                V = pp.tile([P, n_tiles, 1], f32, name=f"v_{nm}")
                nc.vector.tensor_reduce(V[:], DIF[:], mybir.AxisListType.X,
                                        Alu.add)
                return V

            VP = neighbor_value(+1, 35.5, -36.0, "p")
            VM = neighbor_value(-1, -0.5, +36.0, "m")

            NUM = pp.tile([P, n_tiles, 1], f32)
            nc.vector.tensor_tensor(NUM[:], VP[:], VM[:], Alu.subtract)
            SUMN = pp.tile([P, n_tiles, 1], f32)
            nc.vector.tensor_tensor(SUMN[:], VP[:], VM[:], Alu.add)
            DEN = pp.tile([P, n_tiles, 1], f32)
            nc.vector.tensor_scalar(DEN[:], VMAX[:], 2.0, None, Alu.mult)
            nc.vector.tensor_tensor(DEN[:], DEN[:], SUMN[:], Alu.subtract)
            RECD = pp.tile([P, n_tiles, 1], f32)
            SCD = pp.tile([P, n_tiles, 1], f32)
            nc.vector.reciprocal_approx_accurate(RECD[:], DEN[:], SCD[:])
            REF = pp.tile([P, n_tiles, 1], f32)
            nc.vector.scalar_tensor_tensor(
                out=REF[:], in0=NUM[:], scalar=0.5, in1=RECD[:],
                op0=Alu.mult, op1=Alu.mult)
            nc.vector.tensor_tensor(REF[:], IDX[:], REF[:], Alu.add)
            nc.vector.tensor_scalar(ANG[:], REF[:, :, 0], -2.0 * PI / NBINS,
                                    PI, Alu.mult, Alu.add)

            out_view = out_t[:].rearrange("(t p) -> p t", p=P)
            nc.sync.dma_start(out_view, ANG[:])

    nc.compile()
    return nc


def _get_built(b_core, smooth_w, wk_is_ones):
    key = (b_core, tuple(float(x) for x in smooth_w), bool(wk_is_ones))
    if key not in _BUILD_CACHE:
        _BUILD_CACHE[key] = _build(b_core, smooth_w, wk_is_ones)
    return _BUILD_CACHE[key]


# --------------------------------------------------------------------------
# host entry point
# --------------------------------------------------------------------------
def kernel(patch, weight_kernel, smooth_w):
    from concourse import bass_utils

    patch = np.ascontiguousarray(np.asarray(patch, dtype=np.float32))
    weight_kernel = np.asarray(weight_kernel, dtype=np.float32)
    smooth_w = np.asarray(smooth_w, dtype=np.float32)

    B = patch.shape[0]
    assert B % (N_CORES * P) == 0, f"B={B} not divisible by {N_CORES * P}"
    b_core = B // N_CORES
    n_tiles = b_core // P

    wk_is_ones = bool(np.all(weight_kernel == 1.0))
    nc = _get_built(b_core, smooth_w, wk_is_ones)

    x = patch.reshape(N_CORES, b_core, HW)

    iota = np.tile(np.arange(NBINS, dtype=np.float32), n_tiles)
    consts_row = np.concatenate([iota, iota - 64.0]).astype(np.float32)
    consts = np.ascontiguousarray(
        np.broadcast_to(consts_row, (P, consts_row.size)))

    in_maps = []
    for i in range(N_CORES):
        m = {"patch": np.ascontiguousarray(x[i]), "consts": consts}
        if not wk_is_ones:
            m["wk"] = np.ascontiguousarray(
                np.broadcast_to(weight_kernel.reshape(-1), (P, HW)))
        in_maps.append(m)

    res = bass_utils.run_bass_kernel_spmd(nc, in_maps,
                                          core_ids=list(range(N_CORES)))
    out = np.concatenate([r["angle"] for r in res.results])
    return out.astype(np.float32)


# revision 31
# speedup vs baseline: 1.2990x; 1.0915x over previous
"""Trainium2 Bass kernel for CustomizablePatchDominantGradientOrientation.

Pipeline per patch (32x32, fp32):
  sobel (replicate pad, [1,2,1]x[-1,0,1] separable; /8 dropped - the final
  angle is invariant to a global scale on (gx, gy, mag))
  mag = sqrt(gx^2+gy^2+eps'), theta = 2*atan(gy/(mag+gx))  (half-angle atan2)
  36-bin soft histogram via 18 dual-window custom-DVE passes: pass i
  (boundary b = (2i-18)*pi/36) emits in ONE instruction both
    W[2i]  = sum m*clamp(a-b, 0, d)    (in-pipe prefix-sum, streamed to a
                                        stride-0 out AP; final write = total)
    W2[2i] = sum m*clamp(a-b, 0, 2d)   (accumulator out_a path)
  with W[2i+1] = W2[2i] - W[2i]; hist[k] = W[k-1] - W[k] (bounded masks ->
  accumulation roundoff stays tiny).  Then circular [w0,w1,w2] smoothing,
  argmax, parabolic refinement -> angle.

Data parallel: B=32768 patches sharded over 8 NeuronCores (4096 each);
per core 32 tiles of [128 patches x 1024 pixels].  Layout is patch-major:
partitions = patches, free axis = pixels.
"""

import math

import numpy as np

NBINS = 36
PI = math.pi
PATCH = 32
HW = PATCH * PATCH
P = 128          # partitions (patches per tile)
N_CORES = 8
GROUP = 4        # tiles per ACT-table-set phase group
DELTA = PI / 36.0  # window width in atan units (theta = 2a, bin = 2pi/36)

_BUILD_CACHE = {}
_OPS_REGISTERED = {}
NBLK = 8         # 128-px blocks per patch-tile (4 patch rows each)


def _build_sobel_w():
    """lhsT matrices for the PE sobel: gxT_b = sum_s A_s @ TB_{b+s} with
    A_gx_s = kron(V4_s, Hdiff), A_gy_s = kron(D4_s, Hsmooth); boundary
    blocks (b=0/7) get replicate-pad variants of the s=0 factor.
    Returns (warr [NW,128,128] lhsT-ordered, windex {(gxy,kind,s): idx})."""
    V = np.zeros((32, 32), np.float32)
    for r in range(32):
        for dr in (-1, 0, 1):
            rr = min(max(r + dr, 0), 31)
            V[r, rr] += 2.0 if dr == 0 else 1.0
    D = np.zeros((32, 32), np.float32)
    for r in range(32):
        D[r, min(r + 1, 31)] += 1.0
        D[r, max(r - 1, 0)] -= 1.0

    def blk(M, b, s):
        return M[4 * b:4 * b + 4, 4 * (b + s):4 * (b + s) + 4]

    wlist, windex = [], {}
    for name, Vert, Horz in (("gx", V, D), ("gy", D, V)):
        for kind, b in (("first", 0), ("mid", 3), ("last", 7)):
            for s in (-1, 0, 1):
                if b + s < 0 or b + s >= 8:
                    continue
                A = np.kron(blk(Vert, b, s), Horz)
                windex[(name, kind, s)] = len(wlist)
                wlist.append(np.ascontiguousarray(A.T))
    return np.stack(wlist), windex


_SOBEL_W, _SOBEL_WIDX = _build_sobel_w()


# --------------------------------------------------------------------------
# custom DVE ops
# --------------------------------------------------------------------------
def _register_custom_ops():
    """Register the fused ops at runtime (row assignment + sha pin, exactly
    what a source-level `OPS.append` would do).  DBLWIN_ANT's uop program is
    hand-assembled (dual outputs: in-pipe scan -> out port, accum -> out_a);
    its compiled DveOpSpec is pre-seeded into the compile cache."""
    if _OPS_REGISTERED:
        return _OPS_REGISTERED
    from operator import add as _op_add

    import concourse.dve_ops as dve_ops
    from concourse.dve_ops import DveOp, _COMPILE_CACHE
    from concourse.dve_spec import (
        Spec, Src0, Src1, C0, C1, C2, Zero, relu, minn, maxx, lower,
        _has_src1, Bin, _Placement, _State, _Stage, _assemble, COUNT_ONCE,
        PREV, sq as _sq,
    )
    from concourse.dve_uop import AluInp, AluOp, DveOpSpec, OutSel
    from concourse.dve_uop import DveOpSpec

    def _reg(name, spec):
        if name in dve_ops._SUB_OPCODE_FOR_NAME:
            for op in dve_ops.OPS:
                if op.name == name:
                    return op
        row = dve_ops._CUSTOM_DVE_ROW_BASE + len(dve_ops.OPS)
        assert row < 0x20, "custom-DVE row budget exhausted"
        dve_ops._SUB_OPCODE_FOR_NAME[name] = row
        shas = {}
        for ver in ("v3", "v4"):
            s = DveOpSpec(name=name, opcode=row, uops=lower(spec, ver=ver),
                          rd1_en=_has_src1(spec))
            shas[ver] = s.sha(ver)
        op = DveOp(name, spec, subdim=False, uops_sha=shas)
        dve_ops.OPS.append(op)
        dve_ops.CUSTOM_DVE_SPECS[name] = spec
        return op

    def _reg_hand(name, spec, uops):
        """Register with a hand-assembled v3 uop program (bypasses lower())."""
        if name in dve_ops._SUB_OPCODE_FOR_NAME:
            for op in dve_ops.OPS:
                if op.name == name:
                    return op
        row = dve_ops._CUSTOM_DVE_ROW_BASE + len(dve_ops.OPS)
        assert row < 0x20, "custom-DVE row budget exhausted"
        dve_ops._SUB_OPCODE_FOR_NAME[name] = row
        s3 = DveOpSpec(name=name, opcode=row, uops=uops, rd1_en=True)
        op = DveOp(name, spec, subdim=False, uops_sha={"v3": s3.sha("v3")})
        dve_ops.OPS.append(op)
        dve_ops.CUSTOM_DVE_SPECS[name] = spec
        _COMPILE_CACHE[(name, "v3")] = s3
        return op

    def _addmax_ref(in0, in1, s0, s1, imm2):
        return np.maximum(in0 + in1, s0).astype(np.float32)

    def _sqsum_ref(in0, in1, s0, s1, imm2):
        return (in0 * in0 + in1 * in1 + s0).astype(np.float32)

    def _mag_nr_sum_ref(in0, in1, s0, s1, imm2):
        o = (in0 * ((s0 - in0 * in1 * in1 * s1) * in1)).astype(np.float32)
        return o, o.reshape(o.shape[0], -1).sum(axis=-1, keepdims=True)

    def _dblwin_ref(in0, in1, s0, s1, imm2):
        x = (in0 - np.float32(s0)).astype(np.float32)
        u = np.maximum(x, np.float32(0))
        pA = (np.minimum(u, np.float32(imm2)) * in1).astype(np.float32)
        pB = (np.minimum(u, np.float32(s1)) * in1).astype(np.float32)
        out = np.cumsum(pA.astype(np.float64), axis=-1).astype(np.float32)
        acc = (pB.reshape(pB.shape[0], -1).astype(np.float64)
               .sum(axis=-1, keepdims=True).astype(np.float32))
        return out, acc

    _OPS_REGISTERED["addmax"] = _reg(
        "ADD_MAX_ANT",
        Spec(body=maxx(Src0 + Src1, C0), reference=_addmax_ref))
    # g2 = gx^2 + gy^2 + eps
    _OPS_REGISTERED["sqsum"] = _reg(
        "SQ_SUM_ANT",
        Spec(body=_sq(Src0) + _sq(Src1) + C0, reference=_sqsum_ref))
    # m = g2*(c0 - g2*rc^2*c1)*rc (Newton step on the rsqrt seed rc),
    # with sum(m) on the accumulator.
    _OPS_REGISTERED["magnr"] = _reg(
        "MAG_NR_SUM_ANT",
        Spec(body=Src0 * ((C0 - Src0 * _sq(Src1) * C1) * Src1),
             accum=_op_add, reference=_mag_nr_sum_ref))

    # -- DBLWIN_ANT: hand-assembled dual-window clamp-mask reduction --------
    # s0: x = a - swap0(b)          s4: scanA += pA        (lane3 -> out port)
    # s1: u = max(x, swap1(0))      s5: vB = min(lane2(u), swap5(2d))
    # s2: vA = min(u, swap2(d))     s6: pB = vB * m
    # s3: pA = vA * m               s7: accumB += pB       (out_a path)
    SWAP = AluInp.CURR_SWAP_OUT
    u_node = Bin(AluOp.MAX, Src0, Zero)       # capture-lane marker nodes
    scan_node = Bin(AluOp.ADD, Src0, Src1)
    pipeline = [
        _Stage(AluOp.SUBTRACT, Src0, SWAP),
        _Stage(AluOp.MAX, PREV, SWAP),
        _Stage(AluOp.MIN, PREV, SWAP),
        _Stage(AluOp.MULTIPLY, PREV, Src1),
        _Stage(AluOp.ADD, AluInp.CURR_ALU_OUT, PREV),
        _Stage(AluOp.MIN, AluInp.PREV_DELAY_2, SWAP),
        _Stage(AluOp.MULTIPLY, PREV, Src1),
        _Stage(AluOp.ADD, AluInp.CURR_ALU_OUT, PREV),
    ]
    p = _Placement(
        pipeline=pipeline, node_stage={},
        lane={Src0: 0, Src1: 1, u_node: 2, scan_node: 3},
        out_sel=OutSel.DELAY_3, accum_stage=7, captures=[(2, 2), (5, 3)],
    )
    latch_p = _Placement(
        pipeline=[_Stage(AluOp.BYPASS, PREV)] * 8, node_stage={},
        lane={C0: 0, Zero: 1, C2: 2, C1: 3},
        out_sel=OutSel.ALU_OUT, accum_stage=None, captures=[],
    )
    latch_ov = {
        0: _Stage(AluOp.BYPASS, C0, C0, swap=True),
        1: _Stage(AluOp.BYPASS, Zero, Zero, swap=True),
        2: _Stage(AluOp.BYPASS, C2, C2, swap=True),
        4: _Stage(AluOp.BYPASS, Zero, Zero, swap=True),
        5: _Stage(AluOp.BYPASS, C1, C1, swap=True),
        7: _Stage(AluOp.BYPASS, Zero, Zero, swap=True),
    }
    uops = [
        _assemble(_State(placement=latch_p, consume=(False, False),
                         overrides=latch_ov, trigger=COUNT_ONCE, repeat=1,
                         next=(1, 0, 0), write_out=False)),
        _assemble(_State(placement=p, consume=(False, False),
                         overrides={4: _Stage(AluOp.BYPASS, SWAP),
                                    7: _Stage(AluOp.BYPASS, SWAP)},
                         trigger=COUNT_ONCE, repeat=1, next=(2, 0, 0),
                         write_out=False)),
        _assemble(_State(placement=p, consume=(True, True))),
    ]
    _OPS_REGISTERED["dblwin"] = _reg_hand(
        "DBLWIN_ANT",
        Spec(body=minn(maxx(Src0 - C0, Zero), C2) * Src1, accum=_op_add,
             reference=_dblwin_ref),
        uops)
    return _OPS_REGISTERED


# --------------------------------------------------------------------------
# kernel build
# --------------------------------------------------------------------------
def _build(b_core, smooth_w, wk_is_ones):
    import concourse.bacc as bacc
    import concourse.mybir as mybir
    from concourse.tile import TileContext
    from concourse.bass import MemorySpace, broadcast_tensor_aps
    from concourse.masks import make_identity

    ops = _register_custom_ops()
    ADDMAX, SQSUM = ops["addmax"], ops["sqsum"]
    MAGNR, DBLWIN = ops["magnr"], ops["dblwin"]

    f32 = mybir.dt.float32
    Alu = mybir.AluOpType
    Act = mybir.ActivationFunctionType

    n_tiles = b_core // P
    assert b_core % P == 0
    w0, w1, w2 = (float(x) for x in smooth_w)

    nc = bacc.Bacc(None, target_bir_lowering=False, debug=False)
    patch_in = nc.dram_tensor("patch", [b_core, HW], f32, kind="ExternalInput")
    # consts: iota36 repeated n_tiles times, then (iota36 - 64) repeated
    consts_in = nc.dram_tensor("consts", [P, 2 * n_tiles * NBINS], f32,
                               kind="ExternalInput")
    NW = _SOBEL_W.shape[0]
    sobelw_in = nc.dram_tensor("sobelw", [P, NW * P], f32,
                               kind="ExternalInput")
    wk_in = None
    if not wk_is_ones:
        wk_in = nc.dram_tensor("wk", [P, HW], f32, kind="ExternalInput")
    out_t = nc.dram_tensor("angle", [b_core], f32, kind="ExternalOutput")

    with TileContext(nc) as tc:
        with tc.tile_pool(name="pool", bufs=2) as pool, \
             tc.tile_pool(name="persist", bufs=1) as pp, \
             tc.tile_pool(name="psum", bufs=1, space=MemorySpace.PSUM) as pps:
            IOTA = pp.tile([P, n_tiles, NBINS], f32)
            IOTA64 = pp.tile([P, n_tiles, NBINS], f32)
            nc.sync.dma_start(IOTA[:], consts_in[:, 0:n_tiles * NBINS])
            nc.sync.dma_start(IOTA64[:], consts_in[:, n_tiles * NBINS:])
            SW = pp.tile([P, NW, P], f32)
            nc.sync.dma_start(SW[:], sobelw_in[:])
            ID = pp.tile([P, P], f32)
            make_identity(nc, ID[:])
            WK = None
            if wk_in is not None:
                WK = pp.tile([P, HW], f32)
                nc.sync.dma_start(WK[:], wk_in[:])

            WS = pp.tile([P, n_tiles, 18], f32)   # W[2i]  (single window)
            WA = pp.tile([P, n_tiles, 18], f32)   # W2[2i] (double window)
            SMM = pp.tile([P, n_tiles, 1], f32)   # sum(m) per (patch, tile)
            HEXT = pp.tile([P, n_tiles, NBINS + 2], f32)
            ANG = pp.tile([P, n_tiles], f32)

            n_groups = (n_tiles + GROUP - 1) // GROUP
            for g in range(n_groups):
                tiles = range(g * GROUP, min((g + 1) * GROUP, n_tiles))
                slot = {}
                # ---- phase A: sobel, magnitude (sqrt table set) ----
                for t in tiles:
                    s = t % GROUP
                    X = pool.tile([P, HW], f32, tag="x", bufs=3, name=f"x{t}")
                    nc.sync.dma_start(X[:], patch_in[t * P:(t + 1) * P, :])

                    # ---- sobel on the TensorEngine ----
                    # transpose 128-px blocks, banded matmuls against the
                    # kron'd stencil factors, transpose back.
                    TP = pps.tile([P, NBLK, P], f32, tag="tp", bufs=1,
                                  name=f"tp{t}")
                    TBS = pool.tile([P, NBLK, P], f32, tag="tbs", bufs=2,
                                    name=f"tbs{t}")
                    for b in range(NBLK):
                        nc.tensor.transpose(TP[:, b],
                                            X[:, b * P:(b + 1) * P], ID[:])
                        nc.scalar.copy(TBS[:, b], TP[:, b])
                    ACCX = pps.tile([P, NBLK, P], f32, tag="accx", bufs=1,
                                    name=f"accx{t}")
                    ACCY = pps.tile([P, NBLK, P], f32, tag="accy", bufs=1,
                                    name=f"accy{t}")
                    for name, ACC in (("gx", ACCX), ("gy", ACCY)):
                        for b in range(NBLK):
                            kind = ("first" if b == 0 else
                                    "last" if b == NBLK - 1 else "mid")
                            shifts = [sh_ for sh_ in (-1, 0, 1)
                                      if 0 <= b + sh_ < NBLK]
                            for i, sh_ in enumerate(shifts):
                                wi = _SOBEL_WIDX[(name, kind, sh_)]
                                nc.tensor.matmul(
                                    ACC[:, b], SW[:, wi], TBS[:, b + sh_],
                                    start=(i == 0),
                                    stop=(i == len(shifts) - 1))
                    GXT = pool.tile([P, NBLK, P], f32, tag="gxt", bufs=2,
                                    name=f"gxt{t}")
                    GYT = pool.tile([P, NBLK, P], f32, tag="gyt", bufs=2,
                                    name=f"gyt{t}")
                    GX = pool.tile([P, HW], f32, tag=f"gx{s}", bufs=1,
                                   name=f"gx{t}")
                    GY = pool.tile([P, HW], f32, tag=f"gy{s}", bufs=1,
                                   name=f"gy{t}")
                    for b in range(NBLK):
                        nc.scalar.copy(GXT[:, b], ACCX[:, b])
                        nc.scalar.copy(GYT[:, b], ACCY[:, b])
                    TP2 = pps.tile([P, NBLK, P], f32, tag="tpb", bufs=1,
                                   name=f"tp2{t}")
                    for b in range(NBLK):
                        nc.tensor.transpose(TP2[:, b], GXT[:, b], ID[:])
                        nc.scalar.copy(GX[:, b * P:(b + 1) * P], TP2[:, b])
                    TP3 = pps.tile([P, NBLK, P], f32, tag="tpb", bufs=1,
                                   name=f"tp3{t}")
                    for b in range(NBLK):
                        nc.tensor.transpose(TP3[:, b], GYT[:, b], ID[:])
                        nc.scalar.copy(GY[:, b * P:(b + 1) * P], TP3[:, b])

                    if WK is not None:
                        nc.vector.tensor_tensor(GX[:], GX[:], WK[:], Alu.mult)
                        nc.vector.tensor_tensor(GY[:], GY[:], WK[:], Alu.mult)

                    # g2 = gx^2 + gy^2 + eps  (eps scaled by 8^2 vs
                    # reference) - exact DVE fp32 multiplies, one fused op.
                    G2 = pool.tile([P, HW], f32, tag="g2", name=f"g2{t}")
                    nc.vector._custom_dve(SQSUM, out=G2[:], in0=GX[:],
                                          in1=GY[:], s0=6.4e-17)
                    M = pool.tile([P, HW], f32, tag=f"m{s}", bufs=1,
                                   name=f"m{t}")
                    nc.scalar.activation(M[:], G2[:], Act.Sqrt)
                    # one Newton step + final m = g2*nr; sum(m) fused out
                    RC = pool.tile([P, HW], f32, tag="rc", name=f"rc{t}")
                    nc.vector.reciprocal_approx_fast(RC[:], M[:])
                    nc.vector._custom_dve(MAGNR, out=M[:], in0=G2[:],
                                          in1=RC[:], s0=1.5, s1=0.5,
                                          accum_out=SMM[:, t, 0:1])
                    slot[t] = (GX, GY, M)

                # ---- phase B: orientation + histogram (sigmoid table set) --
                for t in tiles:
                    GX, GY, M = slot[t]
                    # d = max(m + gx, 1e-30): the clamp both avoids the
                    # recip(0)=NaN edge and pins rounding-negative d to the
                    # correct wrap side.
                    D = pool.tile([P, HW], f32, tag="g2", name=f"d{t}")
                    nc.vector._custom_dve(ADDMAX, out=D[:], in0=M[:],
                                          in1=GX[:], s0=1e-30)
                    RC = pool.tile([P, HW], f32, tag="rc", name=f"rcb{t}")
                    SC = pool.tile([P, HW], f32, tag="sc", name=f"scb{t}")
                    nc.vector.reciprocal_approx_fast(RC[:], D[:])
                    nc.vector.tensor_tensor(SC[:], GY[:], RC[:], Alu.mult)
                    A = pool.tile([P, HW], f32, tag="a", name=f"a{t}")
                    nc.scalar.activation(A[:], SC[:], Act.Arctan)

                    # 18 dual-window passes: W[2i] via stride-0 scan out,
                    # W2[2i] via accumulator.
                    for i in range(18):
                        b = (2 * i - 18) * PI / 36.0
                        _, cell = broadcast_tensor_aps(A[:],
                                                       WS[:, t, i:i + 1])
                        nc.vector._custom_dve(
                            DBLWIN, out=cell, in0=A[:], in1=M[:],
                            s0=float(b), s1=float(2 * DELTA),
                            imm2=float(DELTA),
                            accum_out=WA[:, t, i:i + 1])

            # ---- tail: assemble hist, smoothing, argmax, refine (batched) --
            # odd bins 1,3..35 -> HEXT cols 2,4..36:  2*WS - WA
            nc.vector.scalar_tensor_tensor(
                out=HEXT[:, :, 2:38:2], in0=WS[:], scalar=2.0, in1=WA[:],
                op0=Alu.mult, op1=Alu.subtract)
            # WAS = WA - WS (kept in WA; WA dead after)
            nc.vector.tensor_tensor(WA[:], WA[:], WS[:], Alu.subtract)
            # even bins 2,4..34 -> HEXT cols 3,5..35: WAS[i] - WS[i+1], i=0..16
            nc.vector.tensor_tensor(
                HEXT[:, :, 3:37:2], WA[:, :, 0:17], WS[:, :, 1:18],
                Alu.subtract)
            # bin 0 -> col 1: delta*sum(m) - WS[0] + WAS[17]
            nc.vector.scalar_tensor_tensor(
                out=HEXT[:, :, 1:2], in0=SMM[:], scalar=float(DELTA),
                in1=WS[:, :, 0:1], op0=Alu.mult, op1=Alu.subtract)
            nc.vector.tensor_tensor(
                HEXT[:, :, 1:2], HEXT[:, :, 1:2], WA[:, :, 17:18], Alu.add)
            # wrap columns
            nc.vector.tensor_copy(HEXT[:, :, 0:1], HEXT[:, :, 36:37])
            nc.vector.tensor_copy(HEXT[:, :, 37:38], HEXT[:, :, 1:2])

            SM = pp.tile([P, n_tiles, NBINS], f32)
            nc.vector.tensor_scalar(SM[:], HEXT[:, :, 2:38], w2, None,
                                    Alu.mult)
            nc.vector.scalar_tensor_tensor(
                out=SM[:], in0=HEXT[:, :, 0:36], scalar=w0, in1=SM[:],
                op0=Alu.mult, op1=Alu.add)
            HS = pp.tile([P, n_tiles, NBINS], f32)
            nc.vector.scalar_tensor_tensor(
                out=HS[:], in0=HEXT[:, :, 1:37], scalar=w1, in1=SM[:],
                op0=Alu.mult, op1=Alu.add)

            VMAX = pp.tile([P, n_tiles, 1], f32)
            nc.vector.tensor_reduce(VMAX[:], HS[:], mybir.AxisListType.X,
                                    Alu.max)
            EQ = pp.tile([P, n_tiles, NBINS], f32)
            hs_b, vmax_b = broadcast_tensor_aps(HS[:], VMAX[:])
            nc.vector.tensor_tensor(EQ[:], hs_b, vmax_b, Alu.is_equal)
            nc.vector.tensor_tensor(EQ[:], EQ[:], IOTA64[:], Alu.mult)
            IDX = pp.tile([P, n_tiles, 1], f32)
            nc.vector.tensor_reduce(IDX[:], EQ[:], mybir.AxisListType.X,
                                    Alu.min)
            nc.vector.tensor_scalar(IDX[:], IDX[:], 64.0, None, Alu.add)

            def neighbor_value(shift, wrap_thr, wrap_add, nm):
                IDXN = pp.tile([P, n_tiles, 1], f32, name=f"idxn_{nm}")
                nc.vector.tensor_scalar(IDXN[:], IDX[:], float(shift), None,
                                        Alu.add)
                WADJ = pp.tile([P, n_tiles, 1], f32, name=f"wadj_{nm}")
                if wrap_add < 0:
                    nc.vector.tensor_scalar(WADJ[:], IDXN[:], wrap_thr,
                                            float(wrap_add), Alu.is_gt,
                                            Alu.mult)
                else:
                    nc.vector.tensor_scalar(WADJ[:], IDXN[:], wrap_thr,
                                            float(wrap_add), Alu.is_lt,
                                            Alu.mult)
                nc.vector.tensor_tensor(IDXN[:], IDXN[:], WADJ[:], Alu.add)
                DIF = pp.tile([P, n_tiles, NBINS], f32, name=f"dif_{nm}")
                iota_b, idxn_b = broadcast_tensor_aps(IOTA[:], IDXN[:])
                nc.vector.tensor_tensor(DIF[:], iota_b, idxn_b, Alu.subtract)
                nc.vector.tensor_scalar(DIF[:], DIF[:], 0.0, None,
                                        Alu.is_equal)
                nc.vector.tensor_tensor(DIF[:], DIF[:], HS[:], Alu.mult)
                V = pp.tile([P, n_tiles, 1], f32, name=f"v_{nm}")
                nc.vector.tensor_reduce(V[:], DIF[:], mybir.AxisListType.X,
                                        Alu.add)
                return V

            VP = neighbor_value(+1, 35.5, -36.0, "p")
            VM = neighbor_value(-1, -0.5, +36.0, "m")

            NUM = pp.tile([P, n_tiles, 1], f32)
            nc.vector.tensor_tensor(NUM[:], VP[:], VM[:], Alu.subtract)
            SUMN = pp.tile([P, n_tiles, 1], f32)
            nc.vector.tensor_tensor(SUMN[:], VP[:], VM[:], Alu.add)
            DEN = pp.tile([P, n_tiles, 1], f32)
            nc.vector.tensor_scalar(DEN[:], VMAX[:], 2.0, None, Alu.mult)
            nc.vector.tensor_tensor(DEN[:], DEN[:], SUMN[:], Alu.subtract)
            RECD = pp.tile([P, n_tiles, 1], f32)
            SCD = pp.tile([P, n_tiles, 1], f32)
            nc.vector.reciprocal_approx_accurate(RECD[:], DEN[:], SCD[:])
            REF = pp.tile([P, n_tiles, 1], f32)
            nc.vector.scalar_tensor_tensor(
                out=REF[:], in0=NUM[:], scalar=0.5, in1=RECD[:],
                op0=Alu.mult, op1=Alu.mult)
            nc.vector.tensor_tensor(REF[:], IDX[:], REF[:], Alu.add)
            nc.vector.tensor_scalar(ANG[:], REF[:, :, 0], -2.0 * PI / NBINS,
                                    PI, Alu.mult, Alu.add)

            out_view = out_t[:].rearrange("(t p) -> p t", p=P)
            nc.sync.dma_start(out_view, ANG[:])

    nc.compile()
    return nc


def _get_built(b_core, smooth_w, wk_is_ones):
    key = (b_core, tuple(float(x) for x in smooth_w), bool(wk_is_ones))
    if key not in _BUILD_CACHE:
        _BUILD_CACHE[key] = _build(b_core, smooth_w, wk_is_ones)
    return _BUILD_CACHE[key]


# --------------------------------------------------------------------------
# host entry point
# --------------------------------------------------------------------------
def kernel(patch, weight_kernel, smooth_w):
    from concourse import bass_utils

    patch = np.ascontiguousarray(np.asarray(patch, dtype=np.float32))
    weight_kernel = np.asarray(weight_kernel, dtype=np.float32)
    smooth_w = np.asarray(smooth_w, dtype=np.float32)

    B = patch.shape[0]
    assert B % (N_CORES * P) == 0, f"B={B} not divisible by {N_CORES * P}"
    b_core = B // N_CORES
    n_tiles = b_core // P

    wk_is_ones = bool(np.all(weight_kernel == 1.0))
    nc = _get_built(b_core, smooth_w, wk_is_ones)

    x = patch.reshape(N_CORES, b_core, HW)

    iota = np.tile(np.arange(NBINS, dtype=np.float32), n_tiles)
    consts_row = np.concatenate([iota, iota - 64.0]).astype(np.float32)
    consts = np.ascontiguousarray(
        np.broadcast_to(consts_row, (P, consts_row.size)))

    NW = _SOBEL_W.shape[0]
    sobelw = np.ascontiguousarray(
        _SOBEL_W.transpose(1, 0, 2).reshape(P, NW * P))

    in_maps = []
    for i in range(N_CORES):
        m = {"patch": np.ascontiguousarray(x[i]), "consts": consts,
             "sobelw": sobelw}
        if not wk_is_ones:
            m["wk"] = np.ascontiguousarray(
                np.broadcast_to(weight_kernel.reshape(-1), (P, HW)))
        in_maps.append(m)

    res = bass_utils.run_bass_kernel_spmd(nc, in_maps,
                                          core_ids=list(range(N_CORES)))
    out = np.concatenate([r["angle"] for r in res.results])
    return out.astype(np.float32)
